# revision 54
# baseline (speedup 1.0000x reference)
"""MobileMQA Trainium2 kernel (8 NeuronCores, SPMD).

Reference computation (per batch b of 2):
  q  = x @ wq + bq                         [1024 tok, 512]
  kv = x @ wkv + bkv                       [1024 tok, 1024]
  kv = depthwise3x3_s2_same(kv) + dw_bias  [256 sp, 1024]
  k, v = split(kv)  -> reshape to shared-KV length M=2048 (channel fold)
  attn = softmax(q @ k^T * 0.125); out = attn @ v
  y = out @ wo + bo

Sharding: core c handles batch b=c//4, query chunk j=c%4 (256 tokens).
KV path (proj+conv) is replicated across the 4 cores of a batch (MQA).

Design notes (cost-model driven):
- All matmul moving operands are bf16 (1.0 cycles/row at any size; fp32r
  pays 4x below 256 rows). PSUM accumulation stays fp32.
- attn@V is computed with exp-scores as the STATIONARY operand and V as
  the moving one: out[l, d] per head accumulates over 16 m-tiles at 64
  moving rows each (16.4k PE cycles vs 32.8k the other way around).
  Softmax denominators come from an extra ones-column matmul per head.
- Depthwise conv runs on PE as 9 diagonal-weight matmuls per 128-channel
  tile, using per-tap valid-rectangle access patterns (no zero-padding
  pass). Diagonal weight matrices are prebuilt on the host.
- All inputs are packed into one bf16 + one f32 DRAM tensor, DMA'd in a
  handful of large chunks ordered by first use (HWDGE issue costs ~650ns
  per dma_start, so few large DMAs beat many small ones).
- Attention is software-pipelined into the kv/conv phase: score matmuls
  for m-tile group t interleave with projection/conv matmuls of group
  t+1, keeping ACT (exp) busy from ~8us onward.
"""
import os
import sys

for _p in ("/opt/trn_rl_repo", "/opt/trn_rl_repo/concourse"):
    if _p not in sys.path:
        sys.path.insert(0, _p)

_TRUNC = int(os.environ.get("KTRUNC", "9"))

import numpy as np
import ml_dtypes

import concourse.bass as bass
import concourse.mybir as mybir
import concourse.tile as tile
from concourse import bacc
from concourse.bass_utils import run_bass_kernel_spmd
from concourse.masks import make_identity

F32 = mybir.dt.float32
F32R = mybir.dt.float32r
BF16 = mybir.dt.bfloat16
AF = mybir.ActivationFunctionType
ALU = mybir.AluOpType
BFNP = ml_dtypes.bfloat16

DIM = 512
NH = 8
HD = 64
B, H, W = 2, 32, 32
L = H * W            # 1024 tokens per batch
KH = KW = 16
NS = KH * KW         # 256 conv-output spatial positions
M = NS * NH          # 2048 shared-KV positions
CH = 2 * DIM         # 1024 kv channels
SCALE = HD ** -0.5   # 0.125

# ---- f32r mega-tensor column layout (segments ordered by first use) ----
# ch-tile processing order: k0 v0 k1 v1 k2 v2 k3 v3 -> c = 0,4,1,5,2,6,3,7
_CORD = [0, 4, 1, 5, 2, 6, 3, 7]
OFF_WKV = {0: 0, 4: 512, 1: 5120, 5: 5632, 2: 6144, 6: 6656,
           3: 7168, 7: 7680}   # per ch-tile c: [4k, 128] = 512 cols
OFF_XT = {0: 1024, 1: 3072}    # [4k, 512] = 2048 per half
NR = 8192

# f32 tensor: cst (bq 4, bo 4, 8 pad), dww [72], bias planes per ch-tile
OFF_DWW = 16
OFF_BPL = {c: 88 + i * 256 for i, c in enumerate(_CORD)}
NF32 = 88 + 8 * 256

# bf16 tensor: q path + wo
OFF_XTC = 0            # [4k, 256] = 1024
OFF_WQ = 1024          # [4t, 4k, 128] = 2048
OFF_WO = 3072          # [4m, 4k, 128] = 2048
NBF = 5120

# conv taps in emission order (all full-rectangle over a zero-padded 33x33
# input layout, SAME padding)
_TAPS = [(0, 0), (0, 1), (0, 2), (1, 0), (1, 1), (1, 2), (2, 0), (2, 1), (2, 2)]
PADW = 33
NPAD = PADW * PADW   # 1089

_NC_CACHE = {}


def _build_program():
    nc = bacc.Bacc(None)

    bigr_d = nc.dram_tensor("bigr", [128, NR], F32R, kind="ExternalInput")
    bigb_d = nc.dram_tensor("bigb", [128, NBF], BF16, kind="ExternalInput")
    bigf_d = nc.dram_tensor("bigf", [128, NF32], F32, kind="ExternalInput")
    y_d = nc.dram_tensor("y", [DIM, 256], F32, kind="ExternalOutput")

    with tile.TileContext(nc) as tc:
        with tc.tile_pool(name="wp", bufs=1) as wp, \
             tc.tile_pool(name="kvsbp", bufs=2) as kvsbp, \
             tc.tile_pool(name="vsbp", bufs=2) as vsbp, \
             tc.tile_pool(name="expp", bufs=6) as expp:

            bigr = wp.tile([128, NR], F32R, tag="bigr")
            bigb = wp.tile([128, NBF], BF16, tag="bigb")
            bigf = wp.tile([128, NF32], F32, tag="bigf")

            def dma_r(lo, hi):
                nc.sync.dma_start(out=bigr[:, lo:hi], in_=bigr_d[:, lo:hi])

            def dma_b(lo, hi):
                nc.sync.dma_start(out=bigb[:, lo:hi], in_=bigb_d[:, lo:hi])

            def dma_f(lo, hi):
                nc.sync.dma_start(out=bigf[:, lo:hi], in_=bigf_d[:, lo:hi])

            dma_f(0, 88)            # cst + dww (tiny, feeds diag build)
            dma_r(0, 1024)          # wkv c0, c4
            dma_r(1024, 3072)       # xT n0
            dma_b(0, 3072)          # xTc + wq
            dma_r(3072, 5120)       # xT n1
            dma_f(88, 600)          # bpl c0, c4
            dma_r(5120, 6144)       # wkv c1, c5
            dma_f(600, 1112)        # bpl c1, c5
            dma_r(6144, 7168)       # wkv c2, c6
            dma_f(1112, NF32)       # bpl rest
            dma_r(7168, NR)         # wkv c3, c7
            dma_b(3072, NBF)        # wo

            identf = wp.tile([128, 128], F32, tag="identf")
            make_identity(nc, identf)
            identb = wp.tile([128, 128], BF16, tag="identb")
            nc.vector.tensor_copy(identb[:, :], identf[:, :])
            onesf = wp.tile([128, 1], F32, tag="onesf")
            nc.vector.memset(onesf, 1.0)
            ones1 = wp.tile([128, 1], BF16, tag="ones1")
            nc.vector.tensor_copy(ones1[:, :], onesf[:, :])
            zpad = wp.tile([128, PADW], F32, tag="zpad")
            nc.vector.memset(zpad, 0.0)
            # diagonal conv-weight matrices, built on DVE with one
            # broadcast tensor_tensor per ch-tile (ident x tap-weight)
            diagr = wp.tile([128, 72 * 128], F32R, tag="diagr")

            def diag_build(c):
                for j in range(9):
                    o = (c * 9 + j) * 128
                    nc.vector.tensor_scalar_mul(
                        diagr[:, o:o + 128], identf[:, :],
                        bigf[:, OFF_DWW + c * 9 + j:OFF_DWW + c * 9 + j + 1])
            # preload the exp ACT table during the DMA window
            warm = wp.tile([1, 1], F32, tag="warm")
            nc.vector.memset(warm, 0.0)
            nc.scalar.activation(warm[:, :], warm[:, :], AF.Exp)

            kT2 = wp.tile([64, M], F32R, tag="kT2")
            qT2 = wp.tile([64, M], F32R, tag="qT2")
            vaug = wp.tile([128, 16 * HD], BF16, tag="vaug")
            attn_sb = wp.tile([128, 2, 512], BF16, tag="attn_sb")
            attnT = wp.tile([128, 4, 256], BF16, tag="attnT")
            zr = wp.tile([128, 16], F32, tag="zr")
            ysb = wp.tile([128, 4, 256], F32, tag="ysb")

            def wkv_l(c, k):
                o = OFF_WKV[c] + k * 128
                return bigr[:, o:o + 128]

            def xt_r(n, k):
                o = OFF_XT[n] + k * 512
                return bigr[:, o:o + 512]

            def diag_l(c, j):
                o = (c * 9 + j) * 128
                return diagr[:, o:o + 128]

            def bpl_v(c):
                o = OFF_BPL[c]
                return bigf[:, o:o + 256]

            with tc.tile_pool(name="stp", bufs=2, space="PSUM") as stp, \
                 tc.tile_pool(name="avpp", bufs=1, space="PSUM") as avpp, \
                 tc.tile_pool(name="zpp", bufs=1, space="PSUM") as zpp:

                avp = [avpp.tile([128, 512], F32, tag=f"avp{l}",
                                 name=f"avp{l}") for l in range(2)]
                zp = zpp.tile([128, 16], F32, tag="zp")

                # ---------- kv proj + conv closures (PSUM pool passed in) ----------
                def new_kvsb(name):
                    """Zero-padded 33x33 conv-input layout; the SAME-pad
                    column (32) and bottom row (32) are zeroed on Pool."""
                    kvsb = kvsbp.tile([128, NPAD], F32R, tag="kvsb", name=name)
                    kb = kvsb[:, :]
                    pad_col = bass.AP(tensor=kb.tensor, offset=kb.offset + 32,
                                      ap=[kb.ap[0], [PADW, PADW]])
                    nc.vector.tensor_copy(pad_col, zpad[:, :])
                    nc.vector.tensor_copy(kvsb[:, PADW * 32:PADW * 32 + 32],
                                          zpad[:, 0:32])
                    return kvsb

                def kv_chunk(scr, c, n, kvsb):
                    kvp = scr.tile([128, 512], F32, tag="scr", name="kvp")
                    for k in range(4):
                        nc.tensor.matmul(kvp[:, :], wkv_l(c, k), xt_r(n, k),
                                         start=(k == 0), stop=(k == 3))
                    # 512 tokens = 16 padded rows of 32
                    kb = kvsb[:, :]
                    dst = bass.AP(tensor=kb.tensor,
                                  offset=kb.offset + n * 16 * PADW,
                                  ap=[kb.ap[0], [PADW, 16], [1, 32]])
                    nc.vector.tensor_copy(dst, kvp[:, :])

                def conv_taps(scr, c, kvsb, taps, cvp=None):
                    if cvp is None:
                        cvp = scr.tile([128, 512], F32, tag="scr", name="cvp")
                    kb = kvsb[:, :]
                    for dy, dx in taps:
                        win = bass.AP(tensor=kb.tensor,
                                      offset=kb.offset + PADW * dy + dx,
                                      ap=[kb.ap[0], [2 * PADW, KH], [2, KW]])
                        nc.tensor.matmul(cvp[:, 0:256], diag_l(c, 3 * dy + dx),
                                         win,
                                         start=((dy, dx) == _TAPS[0]),
                                         stop=((dy, dx) == _TAPS[-1]))
                    return cvp

                def k_finish(t, cvp):
                    # conv out + bias plane -> kT2 (shared K head, bf16)
                    for gi in range(2):
                        g = 2 * t + gi
                        nc.vector.scalar_tensor_tensor(
                            kT2[:, g * 256:(g + 1) * 256],
                            cvp[gi * 64:(gi + 1) * 64, 0:256], 1.0,
                            bpl_v(t)[gi * 64:(gi + 1) * 64, :],
                            op0=ALU.mult, op1=ALU.add)

                def v_finish(t, cvp, vtp):
                    # transpose [128 ch, 128 s] -> [128 s, 128 ch]; the two
                    # 64-wide ch-groups then scatter to their m-tiles
                    c = 4 + t
                    vsb = vsbp.tile([128, 256], BF16, tag="vsb")
                    nc.vector.scalar_tensor_tensor(
                        vsb[:, :], cvp[:, 0:256], 1.0, bpl_v(c)[:, :],
                        op0=ALU.mult, op1=ALU.add)
                    for sh in range(2):
                        vt = vtp.tile([128, 128], BF16, tag="vt")
                        nc.tensor.matmul(vt[:, :],
                                         vsb[:, sh * 128:(sh + 1) * 128],
                                         identb[:, :], is_transpose=True)
                        av = vaug[:, :]
                        dst = bass.AP(tensor=av.tensor,
                                      offset=av.offset + t * 256 + sh * 64,
                                      ap=[av.ap[0], [128, 2], [1, 64]])
                        nc.vector.tensor_copy(dst, vt[:, :])

                def qproj_one(scr, t):
                    qp = scr.tile([128, 512], F32, tag="scr", name="qp")
                    for k in range(4):
                        nc.tensor.matmul(
                            qp[:, 0:256],
                            bigb[:, OFF_WQ + t * 512 + k * 128:
                                 OFF_WQ + t * 512 + (k + 1) * 128],
                            bigb[:, OFF_XTC + k * 256:OFF_XTC + (k + 1) * 256],
                            start=(k == 0), stop=(k == 3))
                    for gi in range(2):
                        h = 2 * t + gi
                        nc.vector.tensor_scalar_add(
                            qT2[:, h * 256:(h + 1) * 256],
                            qp[gi * 64:(gi + 1) * 64, 0:256],
                            bigf[gi * 64:(gi + 1) * 64, t:t + 1])

                def kvconv_thunks(scr, vtp, t):
                    """10 thunks: k-tile t then v-tile t."""
                    st8 = {}

                    def kt_chunk(n):
                        if "k" not in st8:
                            st8["k"] = new_kvsb("kvsbk")
                        kv_chunk(scr, t, n, st8["k"])

                    def kt_conv(lo):
                        if "kc" not in st8:
                            st8["kc"] = conv_taps(scr, t, st8["k"], _TAPS[:4])
                        else:
                            conv_taps(scr, t, st8["k"], _TAPS[4:], st8["kc"])

                    def vt_chunk(n):
                        if "v" not in st8:
                            st8["v"] = new_kvsb("kvsbv")
                        kv_chunk(scr, 4 + t, n, st8["v"])

                    def vt_conv(lo):
                        if "vc" not in st8:
                            st8["vc"] = conv_taps(scr, 4 + t, st8["v"], _TAPS[:4])
                        else:
                            conv_taps(scr, 4 + t, st8["v"], _TAPS[4:], st8["vc"])

                    return [
                        lambda: (diag_build(t), diag_build(4 + t)),
                        lambda: kt_chunk(0),
                        lambda: kt_chunk(1),
                        lambda: kt_conv(0),
                        lambda: kt_conv(4),
                        lambda: k_finish(t, st8["kc"]),
                        lambda: vt_chunk(0),
                        lambda: vt_chunk(1),
                        lambda: vt_conv(0),
                        lambda: vt_conv(4),
                        lambda: (v_finish(t, st8["vc"], vtp)),
                    ]

                # ---------- attention chunk emission ----------
                qv = qT2[:, :].rearrange("p (h l) -> p h l", l=256)

                def emit_S(c):
                    mt, lh, hg = c
                    st = stp.tile([128, 512], F32, tag="st")
                    rhs = qv[:, hg * 4:(hg + 1) * 4, lh * 128:(lh + 1) * 128]
                    nc.tensor.matmul(st[:, :], kT2[:, mt * 128:(mt + 1) * 128],
                                     rhs, start=True, stop=True)
                    ex = expp.tile([128, 512], BF16, tag="ex")
                    nc.scalar.activation(ex[:, :], st[:, :], AF.Exp,
                                         scale=float(SCALE))
                    return ex

                def emit_AV(c, ex):
                    # One PSUM accumulation group per bank: the first matmul
                    # start=True lazily zeroes the whole 2KB region; each
                    # head's first write consumes its share of the zeroing.
                    mt, lh, hg = c
                    for hi in range(4):
                        h = hg * 4 + hi
                        exh = ex[:, hi * 128:(hi + 1) * 128]
                        nc.tensor.matmul(avp[lh][:, h * 64:(h + 1) * 64],
                                         exh, vaug[:, mt * 64:(mt + 1) * 64],
                                         start=(mt == 0 and h == 0),
                                         stop=(mt == 15 and h == 7))
                        nc.tensor.matmul(zp[:, lh * 8 + h:lh * 8 + h + 1],
                                         exh, ones1[:, :],
                                         start=(mt == 0 and lh == 0 and h == 0),
                                         stop=(mt == 15 and lh == 1 and h == 7))

                # ---------- tail closures ----------
                def norm_lh(lh):
                    for h in range(NH):
                        nc.vector.tensor_scalar_mul(
                            attn_sb[:, lh, h * 64:(h + 1) * 64],
                            avp[lh][:, h * 64:(h + 1) * 64],
                            zr[:, lh * 8 + h:lh * 8 + h + 1])

                def tr_lh(lh, trp):
                    for kk in range(4):
                        trt = trp.tile([128, 128], BF16, tag="trp")
                        nc.tensor.matmul(
                            trt[:, :],
                            attn_sb[:, lh, kk * 128:(kk + 1) * 128],
                            identb[:, :], is_transpose=True)
                        nc.vector.tensor_copy(
                            attnT[:, kk, lh * 128:(lh + 1) * 128], trt[:, :])

                def y_m(m, lh, ypp, dma_engine=None):
                    yp = ypp.tile([128, 256], F32, tag="yp")
                    for k in range(4):
                        nc.tensor.matmul(
                            yp[:, 0:128],
                            bigb[:, OFF_WO + m * 512 + k * 128:
                                 OFF_WO + m * 512 + (k + 1) * 128],
                            attnT[:, k, lh * 128:(lh + 1) * 128],
                            start=(k == 0), stop=(k == 3))
                    nc.vector.tensor_scalar_add(
                        ysb[:, m, lh * 128:(lh + 1) * 128], yp[:, 0:128],
                        bigf[:, 4 + m:5 + m])
                    if dma_engine is not None:
                        dma_engine.dma_start(out=y_d[m * 128:(m + 1) * 128, :],
                                             in_=ysb[:, m, :])

                # ---------- chunk schedule ----------
                # chunks: blocks tau=0..3; tau<3 mt-major, tau=3 lh-major
                chunks = []
                for tau in range(3):
                    for mt in range(4 * tau, 4 * tau + 4):
                        for lh in range(2):
                            for hg in range(2):
                                chunks.append((mt, lh, hg))
                for lh in range(2):
                    for mt in range(12, 16):
                        for hg in range(2):
                            chunks.append((mt, lh, hg))

                LAG = 2
                pend = []

                def run_chunks(lo, hi, fills):
                    for i in range(lo, hi):
                        for th in fills.get(i, ()):
                            th()
                        pend.append((chunks[i], emit_S(chunks[i])))
                        if len(pend) > LAG:
                            c, ex = pend.pop(0)
                            emit_AV(c, ex)

                def flush_pend():
                    while pend:
                        c, ex = pend.pop(0)
                        emit_AV(c, ex)

                FILL_SLOTS = [0, 1, 2, 4, 5, 7, 8, 10, 11, 13, 14]

                with tc.tile_pool(name="scr", bufs=2, space="PSUM") as scr, \
                     tc.tile_pool(name="vtp", bufs=1, space="PSUM") as vtp:
                    # prologue: tile 0 kv+conv, q projection
                    if _TRUNC >= 1:
                        nth = int(os.environ.get("KTHUNKS", "99"))
                        for th in kvconv_thunks(scr, vtp, 0)[:nth]:
                            th()
                    if _TRUNC >= 2:
                        for t in range(4):
                            qproj_one(scr, t)
                    # blocks 0-2 with kvconv fills for t+1
                    if _TRUNC >= 3:
                        for tau in range(3):
                            ths = kvconv_thunks(scr, vtp, tau + 1) \
                                if tau < 3 else []
                            fills = {tau * 16 + s: [ths[j]]
                                     for j, s in enumerate(FILL_SLOTS)} \
                                if ths else {}
                            run_chunks(tau * 16, tau * 16 + 16, fills)

                with tc.tile_pool(name="trp", bufs=2, space="PSUM") as trp, \
                     tc.tile_pool(name="ypp", bufs=1, space="PSUM") as ypp:
                    if _TRUNC >= 4:
                        run_chunks(48, 64, {})
                        flush_pend()
                    if _TRUNC >= 5:
                        nc.vector.reciprocal(zr[:, :], zp[:, :])
                        norm_lh(0)
                        tr_lh(0, trp)
                        norm_lh(1)
                        y_m(0, 0, ypp)
                        y_m(1, 0, ypp)
                        tr_lh(1, trp)
                        y_m(2, 0, ypp)
                        y_m(3, 0, ypp)
                        y_m(0, 1, ypp, nc.sync)
                        y_m(1, 1, ypp, nc.scalar)
                        y_m(2, 1, ypp, nc.sync)
                        y_m(3, 1, ypp, nc.scalar)
                    else:
                        nc.vector.memset(ysb, 0.0)
                        for m in range(4):
                            nc.sync.dma_start(out=y_d[m * 128:(m + 1) * 128, :],
                                              in_=ysb[:, m, :])

    nc.finalize()
    return nc


def _get_program():
    if "nc" not in _NC_CACHE:
        _NC_CACHE["nc"] = _build_program()
    return _NC_CACHE["nc"]


def _host_prep(x, wq, bq, wkv, bkv, dw_kernel, dw_bias, wo, bo):
    """Build the 8 per-core input maps (bigb bf16 + bigf f32)."""
    x = np.ascontiguousarray(np.asarray(x, np.float32))
    wq = np.asarray(wq, np.float32)
    wkv = np.asarray(wkv, np.float32)
    wo = np.asarray(wo, np.float32)
    bq = np.asarray(bq, np.float32)
    bkv = np.asarray(bkv, np.float32)
    dw_bias = np.asarray(dw_bias, np.float32)
    bo = np.asarray(bo, np.float32)
    dww = np.asarray(dw_kernel, np.float32).reshape(9, CH).T.copy()  # [1024, 9]

    # bias plane: dw_bias + bkv * sum(valid taps), SAME padding aware
    oy = np.arange(KH)
    valid_y = (2 * oy[:, None] + np.arange(3)[None, :]) < H      # [16, 3]
    valid_x = valid_y.copy()
    wsum = np.zeros((CH, KH, KW), np.float32)
    for tap in range(9):
        dy, dx = tap // 3, tap % 3
        m2 = np.outer(valid_y[:, dy], valid_x[:, dx]).astype(np.float32)
        wsum += dww[:, tap][:, None, None] * m2[None, :, :]
    bpl = (dw_bias[:, None] + bkv[:, None] * wsum.reshape(CH, NS)).astype(np.float32)

    # ---- shared f32r template (np.float32; PE rounds internally) ----
    tmpl = np.zeros((128, NR), np.float32)
    for c in range(8):
        blk = wkv.reshape(4, 128, 8, 128)[:, :, c, :]          # [k, p, cc]
        tmpl[:, OFF_WKV[c]:OFF_WKV[c] + 512] = \
            blk.transpose(1, 0, 2).reshape(128, 512)

    bigb0 = np.zeros((128, NBF), BFNP)
    wqb = wq.reshape(4, 128, 4, 128)                           # [k, p, t, cc]
    for t in range(4):
        bigb0[:, OFF_WQ + t * 512:OFF_WQ + (t + 1) * 512] = \
            wqb[:, :, t, :].transpose(1, 0, 2).reshape(128, 512).astype(BFNP)
    wob = wo.reshape(4, 128, 4, 128)
    for m in range(4):
        bigb0[:, OFF_WO + m * 512:OFF_WO + (m + 1) * 512] = \
            wob[:, :, m, :].transpose(1, 0, 2).reshape(128, 512).astype(BFNP)

    bigf = np.zeros((128, NF32), np.float32)
    bigf[:, 0:4] = bq.reshape(4, 128).T
    bigf[:, 4:8] = bo.reshape(4, 128).T
    for c in range(8):
        bigf[:, OFF_DWW + c * 9:OFF_DWW + (c + 1) * 9] = \
            dww[c * 128:(c + 1) * 128, :]
        bigf[:, OFF_BPL[c]:OFF_BPL[c] + 256] = bpl[c * 128:(c + 1) * 128, :]

    in_maps = []
    for core in range(8):
        b, j = core // 4, core % 4
        xtb = x[b].reshape(L, DIM).T                            # [512, 1024]
        br = tmpl.copy()
        xa = xtb.reshape(4, 128, 2, 512)                        # [k, p, n, t']
        for n in range(2):
            br[:, OFF_XT[n]:OFF_XT[n] + 2048] = \
                xa[:, :, n, :].transpose(1, 0, 2).reshape(128, 2048)
        bigb = bigb0.copy()
        xc = xtb[:, j * 256:(j + 1) * 256].reshape(4, 128, 256)
        bigb[:, OFF_XTC:OFF_XTC + 1024] = \
            xc.transpose(1, 0, 2).reshape(128, 1024).astype(BFNP)
        in_maps.append({"bigr": br, "bigb": bigb, "bigf": bigf})
    return in_maps


def kernel(**inputs) -> np.ndarray:
    nc = _get_program()
    in_maps = _host_prep(**inputs)
    res = run_bass_kernel_spmd(nc, in_maps, core_ids=list(range(8)))
    out = np.zeros((B, H, W, DIM), np.float32)
    flat = out.reshape(B, L, DIM)
    for c in range(8):
        b, j = c // 4, c % 4
        flat[b, j * 256:(j + 1) * 256, :] = res.results[c]["y"].T
    return out


# revision 58
# speedup vs baseline: 1.0199x; 1.0199x over previous
"""MobileMQA Trainium2 kernel (8 NeuronCores, SPMD).

Reference computation (per batch b of 2):
  q  = x @ wq + bq                         [1024 tok, 512]
  kv = x @ wkv + bkv                       [1024 tok, 1024]
  kv = depthwise3x3_s2_same(kv) + dw_bias  [256 sp, 1024]
  k, v = split(kv)  -> reshape to shared-KV length M=2048 (channel fold)
  attn = softmax(q @ k^T * 0.125); out = attn @ v
  y = out @ wo + bo

Sharding: core c handles batch b=c//4, query chunk j=c%4 (256 tokens).
KV path (proj+conv) is replicated across the 4 cores of a batch (MQA).

Design notes (cost-model driven):
- All matmul moving operands are bf16 (1.0 cycles/row at any size; fp32r
  pays 4x below 256 rows). PSUM accumulation stays fp32.
- attn@V is computed with exp-scores as the STATIONARY operand and V as
  the moving one: out[l, d] per head accumulates over 16 m-tiles at 64
  moving rows each (16.4k PE cycles vs 32.8k the other way around).
  Softmax denominators come from an extra ones-column matmul per head.
- Depthwise conv runs on PE as 9 diagonal-weight matmuls per 128-channel
  tile, using per-tap valid-rectangle access patterns (no zero-padding
  pass). Diagonal weight matrices are prebuilt on the host.
- All inputs are packed into one bf16 + one f32 DRAM tensor, DMA'd in a
  handful of large chunks ordered by first use (HWDGE issue costs ~650ns
  per dma_start, so few large DMAs beat many small ones).
- Attention is software-pipelined into the kv/conv phase: score matmuls
  for m-tile group t interleave with projection/conv matmuls of group
  t+1, keeping ACT (exp) busy from ~8us onward.
"""
import os
import sys

for _p in ("/opt/trn_rl_repo", "/opt/trn_rl_repo/concourse"):
    if _p not in sys.path:
        sys.path.insert(0, _p)

_TRUNC = int(os.environ.get("KTRUNC", "9"))

import numpy as np
import ml_dtypes

import concourse.bass as bass
import concourse.mybir as mybir
import concourse.tile as tile
from concourse import bacc
from concourse.bass_utils import run_bass_kernel_spmd
from concourse.masks import make_identity

F32 = mybir.dt.float32
F32R = mybir.dt.float32r
BF16 = mybir.dt.bfloat16
AF = mybir.ActivationFunctionType
ALU = mybir.AluOpType
BFNP = ml_dtypes.bfloat16

DIM = 512
NH = 8
HD = 64
B, H, W = 2, 32, 32
L = H * W            # 1024 tokens per batch
KH = KW = 16
NS = KH * KW         # 256 conv-output spatial positions
M = NS * NH          # 2048 shared-KV positions
CH = 2 * DIM         # 1024 kv channels
SCALE = HD ** -0.5   # 0.125

# ---- f32r mega-tensor column layout (segments ordered by first use) ----
# ch-tile processing order: k0 v0 k1 v1 k2 v2 k3 v3 -> c = 0,4,1,5,2,6,3,7
_CORD = [0, 4, 1, 5, 2, 6, 3, 7]
OFF_WKV = {0: 0, 4: 512, 1: 5120, 5: 5632, 2: 6144, 6: 6656,
           3: 7168, 7: 7680}   # per ch-tile c: [4k, 128] = 512 cols
OFF_XT = {0: 1024, 1: 3072}    # [4k, 512] = 2048 per half
NR = 8192

# f32 tensor: cst (bq 4, bo 4, 8 pad), dww [72], bias planes per ch-tile
OFF_DWW = 16
OFF_BPL = {c: 88 + i * 256 for i, c in enumerate(_CORD)}
NF32 = 88 + 8 * 256

# bf16 tensor: q path + wo
OFF_XTC = 0            # [4k, 256] = 1024
OFF_WQ = 1024          # [4t, 4k, 128] = 2048
OFF_WO = 3072          # [4m, 4k, 128] = 2048
NBF = 5120

# conv taps in emission order (all full-rectangle over a zero-padded 33x33
# input layout, SAME padding)
_TAPS = [(0, 0), (0, 1), (0, 2), (1, 0), (1, 1), (1, 2), (2, 0), (2, 1), (2, 2)]
PADW = 33
NPAD = PADW * PADW   # 1089

_NC_CACHE = {}


def _build_program():
    nc = bacc.Bacc(None)

    bigr_d = nc.dram_tensor("bigr", [128, NR], F32R, kind="ExternalInput")
    bigb_d = nc.dram_tensor("bigb", [128, NBF], BF16, kind="ExternalInput")
    bigf_d = nc.dram_tensor("bigf", [128, NF32], F32, kind="ExternalInput")
    y_d = nc.dram_tensor("y", [DIM, 256], F32, kind="ExternalOutput")

    with tile.TileContext(nc) as tc:
        with tc.tile_pool(name="wp", bufs=1) as wp, \
             tc.tile_pool(name="kvsbp", bufs=2) as kvsbp, \
             tc.tile_pool(name="vsbp", bufs=2) as vsbp, \
             tc.tile_pool(name="expp", bufs=6) as expp:

            bigr = wp.tile([128, NR], F32R, tag="bigr")
            bigb = wp.tile([128, NBF], BF16, tag="bigb")
            bigf = wp.tile([128, NF32], F32, tag="bigf")

            def dma_r(lo, hi):
                nc.sync.dma_start(out=bigr[:, lo:hi], in_=bigr_d[:, lo:hi])

            def dma_b(lo, hi):
                nc.sync.dma_start(out=bigb[:, lo:hi], in_=bigb_d[:, lo:hi])

            def dma_f(lo, hi):
                nc.sync.dma_start(out=bigf[:, lo:hi], in_=bigf_d[:, lo:hi])

            dma_f(0, 88)            # cst + dww (tiny, feeds diag build)
            dma_r(0, 1024)          # wkv c0, c4
            dma_r(1024, 3072)       # xT n0
            dma_r(3072, 5120)       # xT n1
            dma_b(0, 2048)          # xTc + wq t0,t1
            dma_f(88, 600)          # bpl c0, c4
            dma_b(2048, 3072)       # wq t2,t3
            dma_r(5120, 6144)       # wkv c1, c5
            dma_f(600, 1112)        # bpl c1, c5
            dma_r(6144, 7168)       # wkv c2, c6
            dma_f(1112, NF32)       # bpl rest
            dma_r(7168, NR)         # wkv c3, c7
            dma_b(3072, NBF)        # wo

            identf = wp.tile([128, 128], F32, tag="identf")
            make_identity(nc, identf)
            identb = wp.tile([128, 128], BF16, tag="identb")
            nc.vector.tensor_copy(identb[:, :], identf[:, :])
            onesf = wp.tile([128, 1], F32, tag="onesf")
            nc.vector.memset(onesf, 1.0)
            ones1 = wp.tile([128, 1], BF16, tag="ones1")
            nc.vector.tensor_copy(ones1[:, :], onesf[:, :])
            zpad = wp.tile([128, PADW], F32, tag="zpad")
            nc.vector.memset(zpad, 0.0)
            # diagonal conv-weight matrices, built on DVE with one
            # broadcast tensor_tensor per ch-tile (ident x tap-weight)
            diagr = wp.tile([128, 72 * 128], F32R, tag="diagr")

            def diag_build(c):
                # on GPSIMD: the Pool engine is otherwise idle
                for j in range(9):
                    o = (c * 9 + j) * 128
                    nc.gpsimd.tensor_scalar_mul(
                        diagr[:, o:o + 128], identf[:, :],
                        bigf[:, OFF_DWW + c * 9 + j:OFF_DWW + c * 9 + j + 1])
            # preload the exp ACT table during the DMA window
            warm = wp.tile([1, 1], F32, tag="warm")
            nc.vector.memset(warm, 0.0)
            nc.scalar.activation(warm[:, :], warm[:, :], AF.Exp)

            kT2 = wp.tile([64, M], F32R, tag="kT2")
            qT2 = wp.tile([64, M], F32R, tag="qT2")
            vaug = wp.tile([128, 16 * HD], BF16, tag="vaug")
            attn_sb = wp.tile([128, 2, 512], BF16, tag="attn_sb")
            attnT = wp.tile([128, 4, 256], BF16, tag="attnT")
            zr = wp.tile([128, 16], F32, tag="zr")
            ysb = wp.tile([128, 4, 256], F32, tag="ysb")

            def wkv_l(c, k):
                o = OFF_WKV[c] + k * 128
                return bigr[:, o:o + 128]

            def xt_r(n, k):
                o = OFF_XT[n] + k * 512
                return bigr[:, o:o + 512]

            def diag_l(c, j):
                o = (c * 9 + j) * 128
                return diagr[:, o:o + 128]

            def bpl_v(c):
                o = OFF_BPL[c]
                return bigf[:, o:o + 256]

            with tc.tile_pool(name="stp", bufs=2, space="PSUM") as stp, \
                 tc.tile_pool(name="avpp", bufs=1, space="PSUM") as avpp, \
                 tc.tile_pool(name="zpp", bufs=1, space="PSUM") as zpp:

                avp = [avpp.tile([128, 512], F32, tag=f"avp{l}",
                                 name=f"avp{l}") for l in range(2)]
                zp = zpp.tile([128, 16], F32, tag="zp")

                # ---------- kv proj + conv closures (PSUM pool passed in) ----------
                def new_kvsb(name):
                    """Zero-padded 33x33 conv-input layout; the SAME-pad
                    column (32) and bottom row (32) are zeroed on Pool."""
                    kvsb = kvsbp.tile([128, NPAD], F32R, tag="kvsb", name=name)
                    kb = kvsb[:, :]
                    pad_col = bass.AP(tensor=kb.tensor, offset=kb.offset + 32,
                                      ap=[kb.ap[0], [PADW, PADW]])
                    nc.vector.tensor_copy(pad_col, zpad[:, :])
                    nc.vector.tensor_copy(kvsb[:, PADW * 32:PADW * 32 + 32],
                                          zpad[:, 0:32])
                    return kvsb

                def kv_chunk(scr, c, n, kvsb):
                    kvp = scr.tile([128, 512], F32, tag="scr", name="kvp")
                    for k in range(4):
                        nc.tensor.matmul(kvp[:, :], wkv_l(c, k), xt_r(n, k),
                                         start=(k == 0), stop=(k == 3))
                    # 512 tokens = 16 padded rows of 32
                    kb = kvsb[:, :]
                    dst = bass.AP(tensor=kb.tensor,
                                  offset=kb.offset + n * 16 * PADW,
                                  ap=[kb.ap[0], [PADW, 16], [1, 32]])
                    nc.vector.tensor_copy(dst, kvp[:, :])

                def conv_taps(scr, c, kvsb, taps, cvp=None):
                    if cvp is None:
                        cvp = scr.tile([128, 512], F32, tag="scr", name="cvp")
                    kb = kvsb[:, :]
                    for dy, dx in taps:
                        win = bass.AP(tensor=kb.tensor,
                                      offset=kb.offset + PADW * dy + dx,
                                      ap=[kb.ap[0], [2 * PADW, KH], [2, KW]])
                        nc.tensor.matmul(cvp[:, 0:256], diag_l(c, 3 * dy + dx),
                                         win,
                                         start=((dy, dx) == _TAPS[0]),
                                         stop=((dy, dx) == _TAPS[-1]))
                    return cvp

                def k_finish(t, cvp):
                    # conv out + bias plane -> kT2 (shared K head, bf16)
                    for gi in range(2):
                        g = 2 * t + gi
                        nc.vector.scalar_tensor_tensor(
                            kT2[:, g * 256:(g + 1) * 256],
                            cvp[gi * 64:(gi + 1) * 64, 0:256], 1.0,
                            bpl_v(t)[gi * 64:(gi + 1) * 64, :],
                            op0=ALU.mult, op1=ALU.add)

                def v_finish(t, cvp, vtp):
                    # transpose [128 ch, 128 s] -> [128 s, 128 ch]; the two
                    # 64-wide ch-groups then scatter to their m-tiles
                    c = 4 + t
                    vsb = vsbp.tile([128, 256], BF16, tag="vsb")
                    nc.vector.scalar_tensor_tensor(
                        vsb[:, :], cvp[:, 0:256], 1.0, bpl_v(c)[:, :],
                        op0=ALU.mult, op1=ALU.add)
                    for sh in range(2):
                        vt = vtp.tile([128, 128], BF16, tag="vt")
                        nc.tensor.matmul(vt[:, :],
                                         vsb[:, sh * 128:(sh + 1) * 128],
                                         identb[:, :], is_transpose=True)
                        av = vaug[:, :]
                        dst = bass.AP(tensor=av.tensor,
                                      offset=av.offset + t * 256 + sh * 64,
                                      ap=[av.ap[0], [128, 2], [1, 64]])
                        nc.vector.tensor_copy(dst, vt[:, :])

                def qproj_one(scr, t):
                    qp = scr.tile([128, 512], F32, tag="scr", name="qp")
                    for k in range(4):
                        nc.tensor.matmul(
                            qp[:, 0:256],
                            bigb[:, OFF_WQ + t * 512 + k * 128:
                                 OFF_WQ + t * 512 + (k + 1) * 128],
                            bigb[:, OFF_XTC + k * 256:OFF_XTC + (k + 1) * 256],
                            start=(k == 0), stop=(k == 3))
                    for gi in range(2):
                        h = 2 * t + gi
                        nc.vector.tensor_scalar_add(
                            qT2[:, h * 256:(h + 1) * 256],
                            qp[gi * 64:(gi + 1) * 64, 0:256],
                            bigf[gi * 64:(gi + 1) * 64, t:t + 1])

                def kvconv_thunks(scr, vtp, t):
                    """10 thunks: k-tile t then v-tile t."""
                    st8 = {}

                    def kt_chunk(n):
                        if "k" not in st8:
                            st8["k"] = new_kvsb("kvsbk")
                        kv_chunk(scr, t, n, st8["k"])

                    def kt_conv(lo):
                        if "kc" not in st8:
                            st8["kc"] = conv_taps(scr, t, st8["k"], _TAPS[:4])
                        else:
                            conv_taps(scr, t, st8["k"], _TAPS[4:], st8["kc"])

                    def vt_chunk(n):
                        if "v" not in st8:
                            st8["v"] = new_kvsb("kvsbv")
                        kv_chunk(scr, 4 + t, n, st8["v"])

                    def vt_conv(lo):
                        if "vc" not in st8:
                            st8["vc"] = conv_taps(scr, 4 + t, st8["v"], _TAPS[:4])
                        else:
                            conv_taps(scr, 4 + t, st8["v"], _TAPS[4:], st8["vc"])

                    return [
                        lambda: (diag_build(t), diag_build(4 + t)),
                        lambda: kt_chunk(0),
                        lambda: kt_chunk(1),
                        lambda: kt_conv(0),
                        lambda: kt_conv(4),
                        lambda: k_finish(t, st8["kc"]),
                        lambda: vt_chunk(0),
                        lambda: vt_chunk(1),
                        lambda: vt_conv(0),
                        lambda: vt_conv(4),
                        lambda: (v_finish(t, st8["vc"], vtp)),
                    ]

                # ---------- attention chunk emission ----------
                qv = qT2[:, :].rearrange("p (h l) -> p h l", l=256)

                def emit_S(c):
                    mt, lh, hg = c
                    st = stp.tile([128, 512], F32, tag="st")
                    rhs = qv[:, hg * 4:(hg + 1) * 4, lh * 128:(lh + 1) * 128]
                    nc.tensor.matmul(st[:, :], kT2[:, mt * 128:(mt + 1) * 128],
                                     rhs, start=True, stop=True)
                    ex = expp.tile([128, 512], BF16, tag="ex")
                    nc.scalar.activation(ex[:, :], st[:, :], AF.Exp,
                                         scale=float(SCALE))
                    return ex

                def emit_AV(c, ex):
                    # One PSUM accumulation group per bank: the first matmul
                    # start=True lazily zeroes the whole 2KB region; each
                    # head's first write consumes its share of the zeroing.
                    mt, lh, hg = c
                    for hi in range(4):
                        h = hg * 4 + hi
                        exh = ex[:, hi * 128:(hi + 1) * 128]
                        nc.tensor.matmul(avp[lh][:, h * 64:(h + 1) * 64],
                                         exh, vaug[:, mt * 64:(mt + 1) * 64],
                                         start=(mt == 0 and h == 0),
                                         stop=(mt == 15 and h == 7))
                        nc.tensor.matmul(zp[:, lh * 8 + h:lh * 8 + h + 1],
                                         exh, ones1[:, :],
                                         start=(mt == 0 and lh == 0 and h == 0),
                                         stop=(mt == 15 and lh == 1 and h == 7))

                # ---------- tail closures ----------
                def norm_lh(lh):
                    for h in range(NH):
                        nc.vector.tensor_scalar_mul(
                            attn_sb[:, lh, h * 64:(h + 1) * 64],
                            avp[lh][:, h * 64:(h + 1) * 64],
                            zr[:, lh * 8 + h:lh * 8 + h + 1])

                def tr_lh(lh, trp):
                    for kk in range(4):
                        trt = trp.tile([128, 128], BF16, tag="trp")
                        nc.tensor.matmul(
                            trt[:, :],
                            attn_sb[:, lh, kk * 128:(kk + 1) * 128],
                            identb[:, :], is_transpose=True)
                        nc.vector.tensor_copy(
                            attnT[:, kk, lh * 128:(lh + 1) * 128], trt[:, :])

                def y_m(m, lh, ypp, dma_engine=None):
                    yp = ypp.tile([128, 256], F32, tag="yp")
                    for k in range(4):
                        nc.tensor.matmul(
                            yp[:, 0:128],
                            bigb[:, OFF_WO + m * 512 + k * 128:
                                 OFF_WO + m * 512 + (k + 1) * 128],
                            attnT[:, k, lh * 128:(lh + 1) * 128],
                            start=(k == 0), stop=(k == 3))
                    nc.vector.tensor_scalar_add(
                        ysb[:, m, lh * 128:(lh + 1) * 128], yp[:, 0:128],
                        bigf[:, 4 + m:5 + m])
                    if dma_engine is not None:
                        dma_engine.dma_start(out=y_d[m * 128:(m + 1) * 128, :],
                                             in_=ysb[:, m, :])

                # ---------- chunk schedule ----------
                # chunks: blocks tau=0..3; tau<3 mt-major, tau=3 lh-major
                chunks = []
                for tau in range(3):
                    for mt in range(4 * tau, 4 * tau + 4):
                        for lh in range(2):
                            for hg in range(2):
                                chunks.append((mt, lh, hg))
                for lh in range(2):
                    for mt in range(12, 16):
                        for hg in range(2):
                            chunks.append((mt, lh, hg))

                LAG = 2
                pend = []

                def run_chunks(lo, hi, fills):
                    for i in range(lo, hi):
                        for th in fills.get(i, ()):
                            th()
                        pend.append((chunks[i], emit_S(chunks[i])))
                        if len(pend) > LAG:
                            c, ex = pend.pop(0)
                            emit_AV(c, ex)

                def flush_pend():
                    while pend:
                        c, ex = pend.pop(0)
                        emit_AV(c, ex)

                FILL_SLOTS = [0, 1, 2, 4, 5, 7, 8, 10, 11, 13, 14]

                with tc.tile_pool(name="scr", bufs=2, space="PSUM") as scr, \
                     tc.tile_pool(name="vtp", bufs=1, space="PSUM") as vtp:
                    # prologue: tile 0 kv+conv, q projection wedged between
                    # the k- and v-halves so attention can start early
                    if _TRUNC >= 1:
                        ths0 = kvconv_thunks(scr, vtp, 0)
                        for th in ths0[:6]:
                            th()
                    if _TRUNC >= 2:
                        qproj_one(scr, 0)
                        qproj_one(scr, 1)
                    if _TRUNC >= 1:
                        for th in ths0[6:]:
                            th()
                    if _TRUNC >= 2:
                        qproj_one(scr, 2)
                        qproj_one(scr, 3)
                    # blocks 0-2 with kvconv fills for t+1
                    if _TRUNC >= 3:
                        for tau in range(3):
                            ths = kvconv_thunks(scr, vtp, tau + 1) \
                                if tau < 3 else []
                            fills = {tau * 16 + s: [ths[j]]
                                     for j, s in enumerate(FILL_SLOTS)} \
                                if ths else {}
                            run_chunks(tau * 16, tau * 16 + 16, fills)

                with tc.tile_pool(name="trp", bufs=2, space="PSUM") as trp, \
                     tc.tile_pool(name="ypp", bufs=1, space="PSUM") as ypp:
                    if _TRUNC >= 4:
                        run_chunks(48, 64, {})
                        flush_pend()
                    if _TRUNC >= 5:
                        nc.vector.reciprocal(zr[:, :], zp[:, :])
                        norm_lh(0)
                        norm_lh(1)
                        tr_lh(0, trp)
                        y_m(0, 0, ypp)
                        y_m(1, 0, ypp)
                        tr_lh(1, trp)
                        y_m(2, 0, ypp)
                        y_m(3, 0, ypp)
                        y_m(0, 1, ypp, nc.sync)
                        y_m(1, 1, ypp, nc.scalar)
                        y_m(2, 1, ypp, nc.sync)
                        y_m(3, 1, ypp, nc.scalar)
                    else:
                        nc.vector.memset(ysb, 0.0)
                        for m in range(4):
                            nc.sync.dma_start(out=y_d[m * 128:(m + 1) * 128, :],
                                              in_=ysb[:, m, :])

    nc.finalize()
    return nc


def _get_program():
    if "nc" not in _NC_CACHE:
        _NC_CACHE["nc"] = _build_program()
    return _NC_CACHE["nc"]


def _host_prep(x, wq, bq, wkv, bkv, dw_kernel, dw_bias, wo, bo):
    """Build the 8 per-core input maps (bigb bf16 + bigf f32)."""
    x = np.ascontiguousarray(np.asarray(x, np.float32))
    wq = np.asarray(wq, np.float32)
    wkv = np.asarray(wkv, np.float32)
    wo = np.asarray(wo, np.float32)
    bq = np.asarray(bq, np.float32)
    bkv = np.asarray(bkv, np.float32)
    dw_bias = np.asarray(dw_bias, np.float32)
    bo = np.asarray(bo, np.float32)
    dww = np.asarray(dw_kernel, np.float32).reshape(9, CH).T.copy()  # [1024, 9]

    # bias plane: dw_bias + bkv * sum(valid taps), SAME padding aware
    oy = np.arange(KH)
    valid_y = (2 * oy[:, None] + np.arange(3)[None, :]) < H      # [16, 3]
    valid_x = valid_y.copy()
    wsum = np.zeros((CH, KH, KW), np.float32)
    for tap in range(9):
        dy, dx = tap // 3, tap % 3
        m2 = np.outer(valid_y[:, dy], valid_x[:, dx]).astype(np.float32)
        wsum += dww[:, tap][:, None, None] * m2[None, :, :]
    bpl = (dw_bias[:, None] + bkv[:, None] * wsum.reshape(CH, NS)).astype(np.float32)

    # ---- shared f32r template (np.float32; PE rounds internally) ----
    tmpl = np.zeros((128, NR), np.float32)
    for c in range(8):
        blk = wkv.reshape(4, 128, 8, 128)[:, :, c, :]          # [k, p, cc]
        tmpl[:, OFF_WKV[c]:OFF_WKV[c] + 512] = \
            blk.transpose(1, 0, 2).reshape(128, 512)

    bigb0 = np.zeros((128, NBF), BFNP)
    wqb = wq.reshape(4, 128, 4, 128)                           # [k, p, t, cc]
    for t in range(4):
        bigb0[:, OFF_WQ + t * 512:OFF_WQ + (t + 1) * 512] = \
            wqb[:, :, t, :].transpose(1, 0, 2).reshape(128, 512).astype(BFNP)
    wob = wo.reshape(4, 128, 4, 128)
    for m in range(4):
        bigb0[:, OFF_WO + m * 512:OFF_WO + (m + 1) * 512] = \
            wob[:, :, m, :].transpose(1, 0, 2).reshape(128, 512).astype(BFNP)

    bigf = np.zeros((128, NF32), np.float32)
    bigf[:, 0:4] = bq.reshape(4, 128).T
    bigf[:, 4:8] = bo.reshape(4, 128).T
    for c in range(8):
        bigf[:, OFF_DWW + c * 9:OFF_DWW + (c + 1) * 9] = \
            dww[c * 128:(c + 1) * 128, :]
        bigf[:, OFF_BPL[c]:OFF_BPL[c] + 256] = bpl[c * 128:(c + 1) * 128, :]

    in_maps = []
    for core in range(8):
        b, j = core // 4, core % 4
        xtb = x[b].reshape(L, DIM).T                            # [512, 1024]
        br = tmpl.copy()
        xa = xtb.reshape(4, 128, 2, 512)                        # [k, p, n, t']
        for n in range(2):
            br[:, OFF_XT[n]:OFF_XT[n] + 2048] = \
                xa[:, :, n, :].transpose(1, 0, 2).reshape(128, 2048)
        bigb = bigb0.copy()
        xc = xtb[:, j * 256:(j + 1) * 256].reshape(4, 128, 256)
        bigb[:, OFF_XTC:OFF_XTC + 1024] = \
            xc.transpose(1, 0, 2).reshape(128, 1024).astype(BFNP)
        in_maps.append({"bigr": br, "bigb": bigb, "bigf": bigf})
    return in_maps


def kernel(**inputs) -> np.ndarray:
    nc = _get_program()
    in_maps = _host_prep(**inputs)
    res = run_bass_kernel_spmd(nc, in_maps, core_ids=list(range(8)))
    out = np.zeros((B, H, W, DIM), np.float32)
    flat = out.reshape(B, L, DIM)
    for c in range(8):
        b, j = c // 4, c % 4
        flat[b, j * 256:(j + 1) * 256, :] = res.results[c]["y"].T
    return out


# revision 61
# speedup vs baseline: 1.0708x; 1.0500x over previous
"""MobileMQA Trainium2 kernel (8 NeuronCores, SPMD).

Reference computation (per batch b of 2):
  q  = x @ wq + bq                         [1024 tok, 512]
  kv = x @ wkv + bkv                       [1024 tok, 1024]
  kv = depthwise3x3_s2_same(kv) + dw_bias  [256 sp, 1024]
  k, v = split(kv)  -> reshape to shared-KV length M=2048 (channel fold)
  attn = softmax(q @ k^T * 0.125); out = attn @ v
  y = out @ wo + bo

Sharding: core c handles batch b=c//4, query chunk j=c%4 (256 tokens).
KV path (proj+conv) is replicated across the 4 cores of a batch (MQA).

Design notes (cost-model driven):
- All matmul moving operands are bf16 (1.0 cycles/row at any size; fp32r
  pays 4x below 256 rows). PSUM accumulation stays fp32.
- attn@V is computed with exp-scores as the STATIONARY operand and V as
  the moving one: out[l, d] per head accumulates over 16 m-tiles at 64
  moving rows each (16.4k PE cycles vs 32.8k the other way around).
  Softmax denominators come from an extra ones-column matmul per head.
- Depthwise conv runs on PE as 9 diagonal-weight matmuls per 128-channel
  tile, using per-tap valid-rectangle access patterns (no zero-padding
  pass). Diagonal weight matrices are prebuilt on the host.
- All inputs are packed into one bf16 + one f32 DRAM tensor, DMA'd in a
  handful of large chunks ordered by first use (HWDGE issue costs ~650ns
  per dma_start, so few large DMAs beat many small ones).
- Attention is software-pipelined into the kv/conv phase: score matmuls
  for m-tile group t interleave with projection/conv matmuls of group
  t+1, keeping ACT (exp) busy from ~8us onward.
"""
import os
import sys

for _p in ("/opt/trn_rl_repo", "/opt/trn_rl_repo/concourse"):
    if _p not in sys.path:
        sys.path.insert(0, _p)

_TRUNC = int(os.environ.get("KTRUNC", "9"))

import numpy as np
import ml_dtypes

import concourse.bass as bass
import concourse.mybir as mybir
import concourse.tile as tile
from concourse import bacc
from concourse.bass_utils import run_bass_kernel_spmd
from concourse.masks import make_identity

F32 = mybir.dt.float32
F32R = mybir.dt.float32r
BF16 = mybir.dt.bfloat16
AF = mybir.ActivationFunctionType
ALU = mybir.AluOpType
BFNP = ml_dtypes.bfloat16

DIM = 512
NH = 8
HD = 64
B, H, W = 2, 32, 32
L = H * W            # 1024 tokens per batch
KH = KW = 16
NS = KH * KW         # 256 conv-output spatial positions
M = NS * NH          # 2048 shared-KV positions
CH = 2 * DIM         # 1024 kv channels
SCALE = HD ** -0.5   # 0.125

# ---- f32r mega-tensor column layout (segments ordered by first use) ----
# ch-tile processing order: k0 v0 k1 v1 k2 v2 k3 v3 -> c = 0,4,1,5,2,6,3,7
_CORD = [0, 4, 1, 5, 2, 6, 3, 7]
OFF_WKV = {0: 0, 4: 512, 1: 5120, 5: 5632, 2: 6144, 6: 6656,
           3: 7168, 7: 7680}   # per ch-tile c: [4k, 128] = 512 cols
OFF_XT = {0: 1024, 1: 3072}    # [4k, 512] = 2048 per half
NR = 8192

# f32 tensor: cst (bq 4, bo 4, 8 pad), dww [72], bias planes per ch-tile
OFF_DWW = 16
OFF_BPL = {c: 88 + i * 256 for i, c in enumerate(_CORD)}
NF32 = 88 + 8 * 256

# bf16 tensor: q path + wo
OFF_XTC = 0            # [4k, 256] = 1024
OFF_WQ = 1024          # [4t, 4k, 128] = 2048
OFF_WO = 3072          # [4m, 4k, 128] = 2048
NBF = 5120

# conv taps in emission order (all full-rectangle over a zero-padded 33x33
# input layout, SAME padding)
_TAPS = [(0, 0), (0, 1), (0, 2), (1, 0), (1, 1), (1, 2), (2, 0), (2, 1), (2, 2)]
PADW = 33
NPAD = PADW * PADW   # 1089

_NC_CACHE = {}


def _build_program():
    nc = bacc.Bacc(None)

    bigr_d = nc.dram_tensor("bigr", [128, NR], F32R, kind="ExternalInput")
    bigb_d = nc.dram_tensor("bigb", [128, NBF], BF16, kind="ExternalInput")
    bigf_d = nc.dram_tensor("bigf", [128, NF32], F32, kind="ExternalInput")
    y_d = nc.dram_tensor("y", [DIM, 256], F32, kind="ExternalOutput")

    with tile.TileContext(nc) as tc:
        with tc.tile_pool(name="wp", bufs=1) as wp, \
             tc.tile_pool(name="kvsbp", bufs=2) as kvsbp, \
             tc.tile_pool(name="vsbp", bufs=2) as vsbp, \
             tc.tile_pool(name="expp", bufs=6) as expp:

            bigr = wp.tile([128, NR], F32R, tag="bigr")
            bigb = wp.tile([128, NBF], BF16, tag="bigb")
            bigf = wp.tile([128, NF32], F32, tag="bigf")

            def dma_r(lo, hi):
                nc.sync.dma_start(out=bigr[:, lo:hi], in_=bigr_d[:, lo:hi])

            def dma_b(lo, hi):
                nc.sync.dma_start(out=bigb[:, lo:hi], in_=bigb_d[:, lo:hi])

            def dma_f(lo, hi):
                nc.sync.dma_start(out=bigf[:, lo:hi], in_=bigf_d[:, lo:hi])

            dma_r(0, 1024)          # wkv c0, c4
            dma_r(1024, 3072)       # xT n0
            dma_r(3072, 5120)       # xT n1
            dma_f(0, 88)            # cst + dww (feeds Pool diag build)
            dma_b(0, 2048)          # xTc + wq t0,t1
            dma_f(88, 600)          # bpl c0, c4
            dma_b(2048, 3072)       # wq t2,t3
            dma_r(5120, 6144)       # wkv c1, c5
            dma_f(600, 1112)        # bpl c1, c5
            dma_r(6144, 7168)       # wkv c2, c6
            dma_f(1112, NF32)       # bpl rest
            dma_r(7168, NR)         # wkv c3, c7
            dma_b(3072, NBF)        # wo

            identf = wp.tile([128, 128], F32, tag="identf")
            make_identity(nc, identf)
            identb = wp.tile([128, 128], BF16, tag="identb")
            nc.vector.tensor_copy(identb[:, :], identf[:, :])
            onesf = wp.tile([128, 1], F32, tag="onesf")
            nc.vector.memset(onesf, 1.0)
            ones1 = wp.tile([128, 1], BF16, tag="ones1")
            nc.vector.tensor_copy(ones1[:, :], onesf[:, :])
            zpad = wp.tile([128, PADW], F32, tag="zpad")
            nc.vector.memset(zpad, 0.0)
            # diagonal conv-weight matrices, built on DVE with one
            # broadcast tensor_tensor per ch-tile (ident x tap-weight)
            diagr = wp.tile([128, 72 * 128], F32R, tag="diagr")

            def diag_build(c):
                # on GPSIMD: the Pool engine is otherwise idle
                for j in range(9):
                    o = (c * 9 + j) * 128
                    nc.gpsimd.tensor_scalar_mul(
                        diagr[:, o:o + 128], identf[:, :],
                        bigf[:, OFF_DWW + c * 9 + j:OFF_DWW + c * 9 + j + 1])
            # preload the exp ACT table during the DMA window
            warm = wp.tile([1, 1], F32, tag="warm")
            nc.vector.memset(warm, 0.0)
            nc.scalar.activation(warm[:, :], warm[:, :], AF.Exp)

            kT2 = wp.tile([64, M], F32R, tag="kT2")
            qT2 = wp.tile([64, M], F32R, tag="qT2")
            vaug = wp.tile([128, 16 * HD], BF16, tag="vaug")
            attn_sb = wp.tile([128, 2, 512], BF16, tag="attn_sb")
            attnT = wp.tile([128, 4, 256], BF16, tag="attnT")
            zr = wp.tile([128, 16], F32, tag="zr")
            ysb = wp.tile([128, 4, 256], F32, tag="ysb")

            def wkv_l(c, k):
                o = OFF_WKV[c] + k * 128
                return bigr[:, o:o + 128]

            def xt_r(n, k):
                o = OFF_XT[n] + k * 512
                return bigr[:, o:o + 512]

            def diag_l(c, j):
                o = (c * 9 + j) * 128
                return diagr[:, o:o + 128]

            def bpl_v(c):
                o = OFF_BPL[c]
                return bigf[:, o:o + 256]

            with tc.tile_pool(name="stp", bufs=2, space="PSUM") as stp, \
                 tc.tile_pool(name="avpp", bufs=1, space="PSUM") as avpp, \
                 tc.tile_pool(name="zpp", bufs=1, space="PSUM") as zpp:

                avp = [avpp.tile([128, 512], F32, tag=f"avp{l}",
                                 name=f"avp{l}") for l in range(2)]
                zp = zpp.tile([128, 16], F32, tag="zp")

                # ---------- kv proj + conv closures (PSUM pool passed in) ----------
                def new_kvsb(name):
                    """Zero-padded 33x33 conv-input layout; the SAME-pad
                    column (32) and bottom row (32) are zeroed on Pool."""
                    kvsb = kvsbp.tile([128, NPAD], F32R, tag="kvsb", name=name)
                    kb = kvsb[:, :]
                    pad_col = bass.AP(tensor=kb.tensor, offset=kb.offset + 32,
                                      ap=[kb.ap[0], [PADW, PADW]])
                    nc.vector.tensor_copy(pad_col, zpad[:, :])
                    nc.vector.tensor_copy(kvsb[:, PADW * 32:PADW * 32 + 32],
                                          zpad[:, 0:32])
                    return kvsb

                def kv_chunk(scr, c, n, kvsb):
                    kvp = scr.tile([128, 512], F32, tag="scr", name="kvp")
                    for k in range(4):
                        nc.tensor.matmul(kvp[:, :], wkv_l(c, k), xt_r(n, k),
                                         start=(k == 0), stop=(k == 3))
                    # 512 tokens = 16 padded rows of 32
                    kb = kvsb[:, :]
                    dst = bass.AP(tensor=kb.tensor,
                                  offset=kb.offset + n * 16 * PADW,
                                  ap=[kb.ap[0], [PADW, 16], [1, 32]])
                    nc.vector.tensor_copy(dst, kvp[:, :])

                def conv_taps(scr, c, kvsb, taps, cvp=None):
                    if cvp is None:
                        cvp = scr.tile([128, 512], F32, tag="scr", name="cvp")
                    kb = kvsb[:, :]
                    for dy, dx in taps:
                        win = bass.AP(tensor=kb.tensor,
                                      offset=kb.offset + PADW * dy + dx,
                                      ap=[kb.ap[0], [2 * PADW, KH], [2, KW]])
                        nc.tensor.matmul(cvp[:, 0:256], diag_l(c, 3 * dy + dx),
                                         win,
                                         start=((dy, dx) == _TAPS[0]),
                                         stop=((dy, dx) == _TAPS[-1]))
                    return cvp

                def k_finish(t, cvp):
                    # conv out + bias plane -> kT2 (shared K head, bf16)
                    for gi in range(2):
                        g = 2 * t + gi
                        nc.vector.scalar_tensor_tensor(
                            kT2[:, g * 256:(g + 1) * 256],
                            cvp[gi * 64:(gi + 1) * 64, 0:256], 1.0,
                            bpl_v(t)[gi * 64:(gi + 1) * 64, :],
                            op0=ALU.mult, op1=ALU.add)

                def v_finish(t, cvp, vtp):
                    # transpose [128 ch, 128 s] -> [128 s, 128 ch]; the two
                    # 64-wide ch-groups then scatter to their m-tiles
                    c = 4 + t
                    vsb = vsbp.tile([128, 256], BF16, tag="vsb")
                    nc.vector.scalar_tensor_tensor(
                        vsb[:, :], cvp[:, 0:256], 1.0, bpl_v(c)[:, :],
                        op0=ALU.mult, op1=ALU.add)
                    for sh in range(2):
                        vt = vtp.tile([128, 128], BF16, tag="vt")
                        nc.tensor.matmul(vt[:, :],
                                         vsb[:, sh * 128:(sh + 1) * 128],
                                         identb[:, :], is_transpose=True)
                        av = vaug[:, :]
                        dst = bass.AP(tensor=av.tensor,
                                      offset=av.offset + t * 256 + sh * 64,
                                      ap=[av.ap[0], [128, 2], [1, 64]])
                        nc.vector.tensor_copy(dst, vt[:, :])

                def qproj_one(scr, t):
                    qp = scr.tile([128, 512], F32, tag="scr", name="qp")
                    for k in range(4):
                        nc.tensor.matmul(
                            qp[:, 0:256],
                            bigb[:, OFF_WQ + t * 512 + k * 128:
                                 OFF_WQ + t * 512 + (k + 1) * 128],
                            bigb[:, OFF_XTC + k * 256:OFF_XTC + (k + 1) * 256],
                            start=(k == 0), stop=(k == 3))
                    for gi in range(2):
                        h = 2 * t + gi
                        nc.vector.tensor_scalar_add(
                            qT2[:, h * 256:(h + 1) * 256],
                            qp[gi * 64:(gi + 1) * 64, 0:256],
                            bigf[gi * 64:(gi + 1) * 64, t:t + 1])

                def kvconv_thunks(scr, vtp, t):
                    """10 thunks: k-tile t then v-tile t."""
                    st8 = {}

                    def kt_chunk(n):
                        if "k" not in st8:
                            st8["k"] = new_kvsb("kvsbk")
                        kv_chunk(scr, t, n, st8["k"])

                    def kt_conv(lo):
                        if "kc" not in st8:
                            st8["kc"] = conv_taps(scr, t, st8["k"], _TAPS[:4])
                        else:
                            conv_taps(scr, t, st8["k"], _TAPS[4:], st8["kc"])

                    def vt_chunk(n):
                        if "v" not in st8:
                            st8["v"] = new_kvsb("kvsbv")
                        kv_chunk(scr, 4 + t, n, st8["v"])

                    def vt_conv(lo):
                        if "vc" not in st8:
                            st8["vc"] = conv_taps(scr, 4 + t, st8["v"], _TAPS[:4])
                        else:
                            conv_taps(scr, 4 + t, st8["v"], _TAPS[4:], st8["vc"])

                    return [
                        lambda: (diag_build(t), diag_build(4 + t)),
                        lambda: kt_chunk(0),
                        lambda: kt_chunk(1),
                        lambda: kt_conv(0),
                        lambda: kt_conv(4),
                        lambda: k_finish(t, st8["kc"]),
                        lambda: vt_chunk(0),
                        lambda: vt_chunk(1),
                        lambda: vt_conv(0),
                        lambda: vt_conv(4),
                        lambda: (v_finish(t, st8["vc"], vtp)),
                    ]

                # ---------- attention chunk emission ----------
                qv = qT2[:, :].rearrange("p (h l) -> p h l", l=256)

                def emit_S(c):
                    mt, lh, hg = c
                    st = stp.tile([128, 512], F32, tag="st")
                    rhs = qv[:, hg * 4:(hg + 1) * 4, lh * 128:(lh + 1) * 128]
                    nc.tensor.matmul(st[:, :], kT2[:, mt * 128:(mt + 1) * 128],
                                     rhs, start=True, stop=True)
                    ex = expp.tile([128, 512], BF16, tag="ex")
                    nc.scalar.activation(ex[:, :], st[:, :], AF.Exp,
                                         scale=float(SCALE))
                    return ex

                def emit_AV(c, ex):
                    # One PSUM accumulation group per bank: the first matmul
                    # start=True lazily zeroes the whole 2KB region; each
                    # head's first write consumes its share of the zeroing.
                    mt, lh, hg = c
                    for hi in range(4):
                        h = hg * 4 + hi
                        exh = ex[:, hi * 128:(hi + 1) * 128]
                        nc.tensor.matmul(avp[lh][:, h * 64:(h + 1) * 64],
                                         exh, vaug[:, mt * 64:(mt + 1) * 64],
                                         start=(mt == 0 and h == 0),
                                         stop=(mt == 15 and h == 7))
                        nc.tensor.matmul(zp[:, lh * 8 + h:lh * 8 + h + 1],
                                         exh, ones1[:, :],
                                         start=(mt == 0 and lh == 0 and h == 0),
                                         stop=(mt == 15 and lh == 1 and h == 7))

                # ---------- tail closures ----------
                def norm_lh(lh):
                    for h in range(NH):
                        nc.vector.tensor_scalar_mul(
                            attn_sb[:, lh, h * 64:(h + 1) * 64],
                            avp[lh][:, h * 64:(h + 1) * 64],
                            zr[:, lh * 8 + h:lh * 8 + h + 1])

                def tr_lh(lh, trp):
                    for kk in range(4):
                        trt = trp.tile([128, 128], BF16, tag="trp")
                        nc.tensor.matmul(
                            trt[:, :],
                            attn_sb[:, lh, kk * 128:(kk + 1) * 128],
                            identb[:, :], is_transpose=True)
                        nc.vector.tensor_copy(
                            attnT[:, kk, lh * 128:(lh + 1) * 128], trt[:, :])

                def y_m(m, lh, ypp, dma_engine=None):
                    yp = ypp.tile([128, 256], F32, tag="yp")
                    for k in range(4):
                        nc.tensor.matmul(
                            yp[:, 0:128],
                            bigb[:, OFF_WO + m * 512 + k * 128:
                                 OFF_WO + m * 512 + (k + 1) * 128],
                            attnT[:, k, lh * 128:(lh + 1) * 128],
                            start=(k == 0), stop=(k == 3))
                    nc.vector.tensor_scalar_add(
                        ysb[:, m, lh * 128:(lh + 1) * 128], yp[:, 0:128],
                        bigf[:, 4 + m:5 + m])
                    if dma_engine is not None:
                        dma_engine.dma_start(out=y_d[m * 128:(m + 1) * 128, :],
                                             in_=ysb[:, m, :])

                # ---------- chunk schedule ----------
                # block 0 is hg-major (so only q heads 0-3 gate the start),
                # blocks 1-2 mt-major, block 3 lh-major (frees lh0 early)
                chunks = []
                for hg in range(2):
                    for mt in range(4):
                        for lh in range(2):
                            chunks.append((mt, lh, hg))
                for tau in range(1, 3):
                    for mt in range(4 * tau, 4 * tau + 4):
                        for lh in range(2):
                            for hg in range(2):
                                chunks.append((mt, lh, hg))
                for lh in range(2):
                    for mt in range(12, 16):
                        for hg in range(2):
                            chunks.append((mt, lh, hg))

                LAG = 4
                pend = []

                def run_chunks(lo, hi, fills):
                    for i in range(lo, hi):
                        for th in fills.get(i, ()):
                            th()
                        pend.append((chunks[i], emit_S(chunks[i])))
                        if len(pend) > LAG:
                            c, ex = pend.pop(0)
                            emit_AV(c, ex)

                def flush_pend():
                    while pend:
                        c, ex = pend.pop(0)
                        emit_AV(c, ex)

                FILL_SLOTS = [0, 1, 2, 4, 5, 7, 8, 10, 11, 13, 14]

                with tc.tile_pool(name="scr", bufs=2, space="PSUM") as scr, \
                     tc.tile_pool(name="vtp", bufs=1, space="PSUM") as vtp:
                    # prologue: k-half of tile 0 + q heads 0-3; the v-half
                    # and remaining q heads ride as fills inside block 0
                    ths0 = kvconv_thunks(scr, vtp, 0)
                    for th in ths0[:6]:
                        th()
                    qproj_one(scr, 0)
                    qproj_one(scr, 1)
                    ths1 = kvconv_thunks(scr, vtp, 1)
                    fills0 = {
                        0: [ths0[6]], 1: [ths0[7]], 2: [ths0[8]],
                        3: [ths0[9]], 4: [ths0[10]],
                        5: [lambda: (qproj_one(scr, 2), qproj_one(scr, 3))],
                        6: [ths1[0]], 7: [ths1[1]], 8: [ths1[2]],
                        9: [ths1[3]], 10: [ths1[4]], 11: [ths1[5]],
                        12: [ths1[6]], 13: [ths1[7]], 14: [ths1[8]],
                        15: [ths1[9], ths1[10]],
                    }
                    run_chunks(0, 16, fills0)
                    # blocks 1-2 with kvconv fills for t+1
                    for tau in range(1, 3):
                        ths = kvconv_thunks(scr, vtp, tau + 1)
                        fills = {tau * 16 + s: [ths[j]]
                                 for j, s in enumerate(FILL_SLOTS)}
                        run_chunks(tau * 16, tau * 16 + 16, fills)

                with tc.tile_pool(name="trp", bufs=2, space="PSUM") as trp:
                    run_chunks(48, 64, {})
                    flush_pend()
                    nc.vector.reciprocal(zr[:, :], zp[:, :])
                    norm_lh(0)
                    norm_lh(1)
                    tr_lh(0, trp)
                    tr_lh(1, trp)

                with tc.tile_pool(name="ypp", bufs=2, space="PSUM") as ypp:
                    y_m(0, 0, ypp)
                    y_m(1, 0, ypp)
                    y_m(2, 0, ypp)
                    y_m(3, 0, ypp)
                    y_m(0, 1, ypp, nc.sync)
                    y_m(1, 1, ypp, nc.scalar)
                    y_m(2, 1, ypp, nc.sync)
                    y_m(3, 1, ypp, nc.scalar)

    nc.finalize()
    return nc


def _get_program():
    if "nc" not in _NC_CACHE:
        _NC_CACHE["nc"] = _build_program()
    return _NC_CACHE["nc"]


def _host_prep(x, wq, bq, wkv, bkv, dw_kernel, dw_bias, wo, bo):
    """Build the 8 per-core input maps (bigb bf16 + bigf f32)."""
    x = np.ascontiguousarray(np.asarray(x, np.float32))
    wq = np.asarray(wq, np.float32)
    wkv = np.asarray(wkv, np.float32)
    wo = np.asarray(wo, np.float32)
    bq = np.asarray(bq, np.float32)
    bkv = np.asarray(bkv, np.float32)
    dw_bias = np.asarray(dw_bias, np.float32)
    bo = np.asarray(bo, np.float32)
    dww = np.asarray(dw_kernel, np.float32).reshape(9, CH).T.copy()  # [1024, 9]

    # bias plane: dw_bias + bkv * sum(valid taps), SAME padding aware
    oy = np.arange(KH)
    valid_y = (2 * oy[:, None] + np.arange(3)[None, :]) < H      # [16, 3]
    valid_x = valid_y.copy()
    wsum = np.zeros((CH, KH, KW), np.float32)
    for tap in range(9):
        dy, dx = tap // 3, tap % 3
        m2 = np.outer(valid_y[:, dy], valid_x[:, dx]).astype(np.float32)
        wsum += dww[:, tap][:, None, None] * m2[None, :, :]
    bpl = (dw_bias[:, None] + bkv[:, None] * wsum.reshape(CH, NS)).astype(np.float32)

    # ---- shared f32r template (np.float32; PE rounds internally) ----
    tmpl = np.zeros((128, NR), np.float32)
    for c in range(8):
        blk = wkv.reshape(4, 128, 8, 128)[:, :, c, :]          # [k, p, cc]
        tmpl[:, OFF_WKV[c]:OFF_WKV[c] + 512] = \
            blk.transpose(1, 0, 2).reshape(128, 512)

    bigb0 = np.zeros((128, NBF), BFNP)
    wqb = wq.reshape(4, 128, 4, 128)                           # [k, p, t, cc]
    for t in range(4):
        bigb0[:, OFF_WQ + t * 512:OFF_WQ + (t + 1) * 512] = \
            wqb[:, :, t, :].transpose(1, 0, 2).reshape(128, 512).astype(BFNP)
    wob = wo.reshape(4, 128, 4, 128)
    for m in range(4):
        bigb0[:, OFF_WO + m * 512:OFF_WO + (m + 1) * 512] = \
            wob[:, :, m, :].transpose(1, 0, 2).reshape(128, 512).astype(BFNP)

    bigf = np.zeros((128, NF32), np.float32)
    bigf[:, 0:4] = bq.reshape(4, 128).T
    bigf[:, 4:8] = bo.reshape(4, 128).T
    for c in range(8):
        bigf[:, OFF_DWW + c * 9:OFF_DWW + (c + 1) * 9] = \
            dww[c * 128:(c + 1) * 128, :]
        bigf[:, OFF_BPL[c]:OFF_BPL[c] + 256] = bpl[c * 128:(c + 1) * 128, :]

    in_maps = []
    for core in range(8):
        b, j = core // 4, core % 4
        xtb = x[b].reshape(L, DIM).T                            # [512, 1024]
        br = tmpl.copy()
        xa = xtb.reshape(4, 128, 2, 512)                        # [k, p, n, t']
        for n in range(2):
            br[:, OFF_XT[n]:OFF_XT[n] + 2048] = \
                xa[:, :, n, :].transpose(1, 0, 2).reshape(128, 2048)
        bigb = bigb0.copy()
        xc = xtb[:, j * 256:(j + 1) * 256].reshape(4, 128, 256)
        bigb[:, OFF_XTC:OFF_XTC + 1024] = \
            xc.transpose(1, 0, 2).reshape(128, 1024).astype(BFNP)
        in_maps.append({"bigr": br, "bigb": bigb, "bigf": bigf})
    return in_maps


def kernel(**inputs) -> np.ndarray:
    nc = _get_program()
    in_maps = _host_prep(**inputs)
    res = run_bass_kernel_spmd(nc, in_maps, core_ids=list(range(8)))
    out = np.zeros((B, H, W, DIM), np.float32)
    flat = out.reshape(B, L, DIM)
    for c in range(8):
        b, j = c // 4, c % 4
        flat[b, j * 256:(j + 1) * 256, :] = res.results[c]["y"].T
    return out


# revision 70
# speedup vs baseline: 1.0888x; 1.0168x over previous
"""MobileMQA Trainium2 kernel (8 NeuronCores, SPMD).

Reference computation (per batch b of 2):
  q  = x @ wq + bq                         [1024 tok, 512]
  kv = x @ wkv + bkv                       [1024 tok, 1024]
  kv = depthwise3x3_s2_same(kv) + dw_bias  [256 sp, 1024]
  k, v = split(kv)  -> reshape to shared-KV length M=2048 (channel fold)
  attn = softmax(q @ k^T * 0.125); out = attn @ v
  y = out @ wo + bo

Sharding: core c handles batch b=c//4, query chunk j=c%4 (256 tokens).
KV path (proj+conv) is replicated across the 4 cores of a batch (MQA).

Design notes (cost-model driven):
- All matmul moving operands are bf16 (1.0 cycles/row at any size; fp32r
  pays 4x below 256 rows). PSUM accumulation stays fp32.
- attn@V is computed with exp-scores as the STATIONARY operand and V as
  the moving one: out[l, d] per head accumulates over 16 m-tiles at 64
  moving rows each (16.4k PE cycles vs 32.8k the other way around).
  Softmax denominators come from an extra ones-column matmul per head.
- Depthwise conv runs on PE as 9 diagonal-weight matmuls per 128-channel
  tile, using per-tap valid-rectangle access patterns (no zero-padding
  pass). Diagonal weight matrices are prebuilt on the host.
- All inputs are packed into one bf16 + one f32 DRAM tensor, DMA'd in a
  handful of large chunks ordered by first use (HWDGE issue costs ~650ns
  per dma_start, so few large DMAs beat many small ones).
- Attention is software-pipelined into the kv/conv phase: score matmuls
  for m-tile group t interleave with projection/conv matmuls of group
  t+1, keeping ACT (exp) busy from ~8us onward.
"""
import os
import sys

for _p in ("/opt/trn_rl_repo", "/opt/trn_rl_repo/concourse"):
    if _p not in sys.path:
        sys.path.insert(0, _p)

_TRUNC = int(os.environ.get("KTRUNC", "9"))

import numpy as np
import ml_dtypes

import concourse.bass as bass
import concourse.mybir as mybir
import concourse.tile as tile
from concourse import bacc
from concourse.bass_utils import run_bass_kernel_spmd
from concourse.masks import make_identity

F32 = mybir.dt.float32
F32R = mybir.dt.float32r
BF16 = mybir.dt.bfloat16
AF = mybir.ActivationFunctionType
ALU = mybir.AluOpType
BFNP = ml_dtypes.bfloat16

DIM = 512
NH = 8
HD = 64
B, H, W = 2, 32, 32
L = H * W            # 1024 tokens per batch
KH = KW = 16
NS = KH * KW         # 256 conv-output spatial positions
M = NS * NH          # 2048 shared-KV positions
CH = 2 * DIM         # 1024 kv channels
SCALE = HD ** -0.5   # 0.125

# ---- f32r mega-tensor column layout (segments ordered by first use) ----
# ch-tile processing order: k0 v0 k1 v1 k2 v2 k3 v3 -> c = 0,4,1,5,2,6,3,7
# cols 0..3072 are k-interleaved [wkv-c0-k | wkv-c4-k | xT-n0-k] so the
# first kv matmul fires as soon as the k0 block lands
_CORD = [0, 4, 1, 5, 2, 6, 3, 7]
OFF_WKV = {1: 5120, 5: 5632, 2: 6144, 6: 6656, 3: 7168, 7: 7680}
OFF_XT1 = 3072                 # xT n1: [4k, 512]
NR = 8192

# conv taps: dy=0,1 run on PE (diag matmuls), dy=2 on DVE (stt chain)
_PE_TAPS = [(0, 0), (0, 1), (0, 2), (1, 0), (1, 1), (1, 2)]
_DVE_TAPS = [(2, 0), (2, 1), (2, 2)]

# f32 tensor: cst (bq 4, bo 4, 8 pad), dww [72], bias planes per ch-tile
OFF_DWW = 16
OFF_BPL = {c: 88 + i * 256 for i, c in enumerate(_CORD)}
NF32 = 88 + 8 * 256

# bf16 tensor: q path + wo
OFF_XTC = 0            # [4k, 256] = 1024
OFF_WQ = 1024          # [4t, 4k, 128] = 2048
OFF_WO = 3072          # [4m, 4k, 128] = 2048
NBF = 5120

PADW = 33
NPAD = PADW * PADW   # 1089

_NC_CACHE = {}


def _build_program():
    nc = bacc.Bacc(None)

    bigr_d = nc.dram_tensor("bigr", [128, NR], F32R, kind="ExternalInput")
    bigb_d = nc.dram_tensor("bigb", [128, NBF], BF16, kind="ExternalInput")
    bigf_d = nc.dram_tensor("bigf", [128, NF32], F32, kind="ExternalInput")
    y_d = nc.dram_tensor("y", [DIM, 256], F32, kind="ExternalOutput")

    with tile.TileContext(nc) as tc:
        with tc.tile_pool(name="wp", bufs=1) as wp, \
             tc.tile_pool(name="kvsbp", bufs=2) as kvsbp, \
             tc.tile_pool(name="vsbp", bufs=2) as vsbp, \
             tc.tile_pool(name="accp", bufs=2) as accp, \
             tc.tile_pool(name="expp", bufs=6) as expp:

            bigr = wp.tile([128, NR], F32R, tag="bigr")
            bigb = wp.tile([128, NBF], BF16, tag="bigb")
            bigf = wp.tile([128, NF32], F32, tag="bigf")

            def dma_r(lo, hi):
                nc.sync.dma_start(out=bigr[:, lo:hi], in_=bigr_d[:, lo:hi])

            def dma_b(lo, hi):
                nc.sync.dma_start(out=bigb[:, lo:hi], in_=bigb_d[:, lo:hi])

            def dma_f(lo, hi):
                nc.sync.dma_start(out=bigf[:, lo:hi], in_=bigf_d[:, lo:hi])

            for k in range(4):      # [wkv c0c4 | xT n0] per k-slice
                dma_r(k * 768, (k + 1) * 768)
            dma_f(0, 88)            # cst + dww (feeds Pool diag build)
            dma_b(0, 2048)          # xTc + wq t0,t1
            dma_r(3072, 5120)       # xT n1
            dma_f(88, 600)          # bpl c0, c4
            dma_b(2048, 3072)       # wq t2,t3
            dma_r(5120, 6144)       # wkv c1, c5
            dma_f(600, 1112)        # bpl c1, c5
            dma_r(6144, 7168)       # wkv c2, c6
            dma_f(1112, NF32)       # bpl rest
            dma_r(7168, NR)         # wkv c3, c7
            dma_b(3072, NBF)        # wo

            identf = wp.tile([128, 128], F32, tag="identf")
            make_identity(nc, identf)
            identb = wp.tile([128, 128], BF16, tag="identb")
            nc.vector.tensor_copy(identb[:, :], identf[:, :])
            onesf = wp.tile([128, 1], F32, tag="onesf")
            nc.vector.memset(onesf, 1.0)
            ones1 = wp.tile([128, 1], BF16, tag="ones1")
            nc.vector.tensor_copy(ones1[:, :], onesf[:, :])
            zpad = wp.tile([128, PADW], F32, tag="zpad")
            nc.vector.memset(zpad, 0.0)
            # diagonal conv-weight matrices, built on DVE with one
            # broadcast tensor_tensor per ch-tile (ident x tap-weight)
            diagr = wp.tile([128, 72 * 128], F32R, tag="diagr")

            def diag_build(c):
                # on GPSIMD: the Pool engine is otherwise idle
                for j in range(9):
                    o = (c * 9 + j) * 128
                    nc.gpsimd.tensor_scalar_mul(
                        diagr[:, o:o + 128], identf[:, :],
                        bigf[:, OFF_DWW + c * 9 + j:OFF_DWW + c * 9 + j + 1])
            # preload the exp ACT table during the DMA window
            warm = wp.tile([1, 1], F32, tag="warm")
            nc.vector.memset(warm, 0.0)
            nc.scalar.activation(warm[:, :], warm[:, :], AF.Exp)

            kT2 = wp.tile([64, M], F32R, tag="kT2")
            qT2 = wp.tile([64, M], F32R, tag="qT2")
            vaug = wp.tile([128, 16 * HD], BF16, tag="vaug")
            attn_sb = wp.tile([128, 2, 512], BF16, tag="attn_sb")
            attnT = wp.tile([128, 4, 256], BF16, tag="attnT")
            zr = wp.tile([128, 16], F32, tag="zr")
            ysb = wp.tile([128, 4, 256], F32, tag="ysb")

            def wkv_l(c, k):
                if c == 0:
                    o = k * 768
                elif c == 4:
                    o = k * 768 + 128
                else:
                    o = OFF_WKV[c] + k * 128
                return bigr[:, o:o + 128]

            def xt_r(n, k):
                o = (k * 768 + 256) if n == 0 else (OFF_XT1 + k * 512)
                return bigr[:, o:o + 512]

            def diag_l(c, j):
                o = (c * 9 + j) * 128
                return diagr[:, o:o + 128]

            def bpl_v(c):
                o = OFF_BPL[c]
                return bigf[:, o:o + 256]

            with tc.tile_pool(name="stp", bufs=2, space="PSUM") as stp, \
                 tc.tile_pool(name="avpp", bufs=1, space="PSUM") as avpp, \
                 tc.tile_pool(name="zpp", bufs=1, space="PSUM") as zpp:

                avp = [avpp.tile([128, 512], F32, tag=f"avp{l}",
                                 name=f"avp{l}") for l in range(2)]
                zp = zpp.tile([128, 16], F32, tag="zp")

                # ---------- kv proj + conv closures (PSUM pool passed in) ----------
                def new_kvsb(name):
                    """Zero-padded 33x33 conv-input layout; the SAME-pad
                    column (32) and bottom row (32) are zeroed on Pool."""
                    kvsb = kvsbp.tile([128, NPAD], F32R, tag="kvsb", name=name)
                    kb = kvsb[:, :]
                    pad_col = bass.AP(tensor=kb.tensor, offset=kb.offset + 32,
                                      ap=[kb.ap[0], [PADW, PADW]])
                    nc.vector.tensor_copy(pad_col, zpad[:, :])
                    nc.vector.tensor_copy(kvsb[:, PADW * 32:PADW * 32 + 32],
                                          zpad[:, 0:32])
                    return kvsb

                def kv_chunk(scr, c, n, kvsb):
                    kvp = scr.tile([128, 512], F32, tag="scr", name="kvp")
                    for k in range(4):
                        nc.tensor.matmul(kvp[:, :], wkv_l(c, k), xt_r(n, k),
                                         start=(k == 0), stop=(k == 3))
                    # 512 tokens = 16 padded rows of 32
                    kb = kvsb[:, :]
                    dst = bass.AP(tensor=kb.tensor,
                                  offset=kb.offset + n * 16 * PADW,
                                  ap=[kb.ap[0], [PADW, 16], [1, 32]])
                    nc.vector.tensor_copy(dst, kvp[:, :])

                def _win(kvsb, dy, dx):
                    kb = kvsb[:, :]
                    return bass.AP(tensor=kb.tensor,
                                   offset=kb.offset + PADW * dy + dx,
                                   ap=[kb.ap[0], [2 * PADW, KH], [2, KW]])

                def conv_dve(c, kvsb):
                    # dy=2 tap row on DVE: acc = bias-plane + 3 taps
                    acc = accp.tile([128, 256], F32, tag="acc")
                    av = acc[:, :].rearrange("p (a b) -> p a b", b=KW)
                    for i, (dy, dx) in enumerate(_DVE_TAPS):
                        prev = bpl_v(c)[:, :].rearrange(
                            "p (a b) -> p a b", b=KW) if i == 0 else av
                        nc.vector.scalar_tensor_tensor(
                            av, _win(kvsb, dy, dx),
                            bigf[:, OFF_DWW + c * 9 + 3 * dy + dx:
                                 OFF_DWW + c * 9 + 3 * dy + dx + 1],
                            prev, op0=ALU.mult, op1=ALU.add)
                    return acc

                def conv_taps(scr, c, kvsb, taps, cvp=None):
                    if cvp is None:
                        cvp = scr.tile([128, 512], F32, tag="scr", name="cvp")
                    for dy, dx in taps:
                        nc.tensor.matmul(cvp[:, 0:256], diag_l(c, 3 * dy + dx),
                                         _win(kvsb, dy, dx),
                                         start=((dy, dx) == _PE_TAPS[0]),
                                         stop=((dy, dx) == _PE_TAPS[-1]))
                    return cvp

                def k_finish(t, cvp, acc):
                    # PE conv partial + DVE partial (incl bias) -> kT2
                    for gi in range(2):
                        g = 2 * t + gi
                        nc.vector.tensor_tensor(
                            kT2[:, g * 256:(g + 1) * 256],
                            cvp[gi * 64:(gi + 1) * 64, 0:256],
                            acc[gi * 64:(gi + 1) * 64, :], op=ALU.add)

                def v_finish(t, cvp, acc, vtp):
                    # transpose [128 ch, 128 s] -> [128 s, 128 ch]; the two
                    # 64-wide ch-groups then scatter to their m-tiles
                    vsb = vsbp.tile([128, 256], BF16, tag="vsb")
                    nc.vector.tensor_tensor(
                        vsb[:, :], cvp[:, 0:256], acc[:, :], op=ALU.add)
                    for sh in range(2):
                        vt = vtp.tile([128, 128], BF16, tag="vt")
                        nc.tensor.matmul(vt[:, :],
                                         vsb[:, sh * 128:(sh + 1) * 128],
                                         identb[:, :], is_transpose=True)
                        av = vaug[:, :]
                        dst = bass.AP(tensor=av.tensor,
                                      offset=av.offset + t * 256 + sh * 64,
                                      ap=[av.ap[0], [128, 2], [1, 64]])
                        nc.vector.tensor_copy(dst, vt[:, :])

                def qproj_one(scr, t):
                    qp = scr.tile([128, 512], F32, tag="scr", name="qp")
                    for k in range(4):
                        nc.tensor.matmul(
                            qp[:, 0:256],
                            bigb[:, OFF_WQ + t * 512 + k * 128:
                                 OFF_WQ + t * 512 + (k + 1) * 128],
                            bigb[:, OFF_XTC + k * 256:OFF_XTC + (k + 1) * 256],
                            start=(k == 0), stop=(k == 3))
                    for gi in range(2):
                        h = 2 * t + gi
                        nc.vector.tensor_scalar_add(
                            qT2[:, h * 256:(h + 1) * 256],
                            qp[gi * 64:(gi + 1) * 64, 0:256],
                            bigf[gi * 64:(gi + 1) * 64, t:t + 1])

                def kvconv_thunks(scr, vtp, t):
                    """10 thunks: k-tile t then v-tile t."""
                    st8 = {}

                    def kt_chunk(n):
                        if "k" not in st8:
                            st8["k"] = new_kvsb("kvsbk")
                        kv_chunk(scr, t, n, st8["k"])

                    def kt_conv(lo):
                        if "kc" not in st8:
                            st8["ka"] = conv_dve(t, st8["k"])
                            st8["kc"] = conv_taps(scr, t, st8["k"],
                                                  _PE_TAPS[:3])
                        else:
                            conv_taps(scr, t, st8["k"], _PE_TAPS[3:],
                                      st8["kc"])

                    def vt_chunk(n):
                        if "v" not in st8:
                            st8["v"] = new_kvsb("kvsbv")
                        kv_chunk(scr, 4 + t, n, st8["v"])

                    def vt_conv(lo):
                        if "vc" not in st8:
                            st8["va"] = conv_dve(4 + t, st8["v"])
                            st8["vc"] = conv_taps(scr, 4 + t, st8["v"],
                                                  _PE_TAPS[:3])
                        else:
                            conv_taps(scr, 4 + t, st8["v"], _PE_TAPS[3:],
                                      st8["vc"])

                    return [
                        lambda: (diag_build(t), diag_build(4 + t)),
                        lambda: kt_chunk(0),
                        lambda: kt_chunk(1),
                        lambda: kt_conv(0),
                        lambda: kt_conv(4),
                        lambda: k_finish(t, st8["kc"], st8["ka"]),
                        lambda: vt_chunk(0),
                        lambda: vt_chunk(1),
                        lambda: vt_conv(0),
                        lambda: vt_conv(4),
                        lambda: (v_finish(t, st8["vc"], st8["va"], vtp)),
                    ]

                # ---------- attention chunk emission ----------
                qv = qT2[:, :].rearrange("p (h l) -> p h l", l=256)

                def emit_S(c):
                    mt, lh, hg = c
                    st = stp.tile([128, 512], F32, tag="st")
                    rhs = qv[:, hg * 4:(hg + 1) * 4, lh * 128:(lh + 1) * 128]
                    nc.tensor.matmul(st[:, :], kT2[:, mt * 128:(mt + 1) * 128],
                                     rhs, start=True, stop=True)
                    ex = expp.tile([128, 512], BF16, tag="ex")
                    nc.scalar.activation(ex[:, :], st[:, :], AF.Exp,
                                         scale=float(SCALE))
                    return ex

                def emit_AV(c, ex):
                    # One PSUM accumulation group per bank: the first matmul
                    # start=True lazily zeroes the whole 2KB region; each
                    # head's first write consumes its share of the zeroing.
                    mt, lh, hg = c
                    for hi in range(4):
                        h = hg * 4 + hi
                        exh = ex[:, hi * 128:(hi + 1) * 128]
                        nc.tensor.matmul(avp[lh][:, h * 64:(h + 1) * 64],
                                         exh, vaug[:, mt * 64:(mt + 1) * 64],
                                         start=(mt == 0 and h == 0),
                                         stop=(mt == 15 and h == 7))
                        nc.tensor.matmul(zp[:, lh * 8 + h:lh * 8 + h + 1],
                                         exh, ones1[:, :],
                                         start=(mt == 0 and lh == 0 and h == 0),
                                         stop=(mt == 15 and lh == 1 and h == 7))

                # ---------- tail closures ----------
                def norm_lh(lh):
                    for h in range(NH):
                        nc.vector.tensor_scalar_mul(
                            attn_sb[:, lh, h * 64:(h + 1) * 64],
                            avp[lh][:, h * 64:(h + 1) * 64],
                            zr[:, lh * 8 + h:lh * 8 + h + 1])

                def tr_lh(lh, trp):
                    for kk in range(4):
                        trt = trp.tile([128, 128], BF16, tag="trp")
                        nc.tensor.matmul(
                            trt[:, :],
                            attn_sb[:, lh, kk * 128:(kk + 1) * 128],
                            identb[:, :], is_transpose=True)
                        nc.vector.tensor_copy(
                            attnT[:, kk, lh * 128:(lh + 1) * 128], trt[:, :])

                def y_m(m, lh, ypp, dma_engine=None):
                    yp = ypp.tile([128, 256], F32, tag="yp")
                    for k in range(4):
                        nc.tensor.matmul(
                            yp[:, 0:128],
                            bigb[:, OFF_WO + m * 512 + k * 128:
                                 OFF_WO + m * 512 + (k + 1) * 128],
                            attnT[:, k, lh * 128:(lh + 1) * 128],
                            start=(k == 0), stop=(k == 3))
                    nc.vector.tensor_scalar_add(
                        ysb[:, m, lh * 128:(lh + 1) * 128], yp[:, 0:128],
                        bigf[:, 4 + m:5 + m])
                    if dma_engine is not None:
                        dma_engine.dma_start(out=y_d[m * 128:(m + 1) * 128, :],
                                             in_=ysb[:, m, :])

                # ---------- chunk schedule ----------
                # block 0 is hg-major (so only q heads 0-3 gate the start),
                # blocks 1-2 mt-major, block 3 lh-major (frees lh0 early)
                chunks = []
                for hg in range(2):
                    for mt in range(4):
                        for lh in range(2):
                            chunks.append((mt, lh, hg))
                for tau in range(1, 3):
                    for mt in range(4 * tau, 4 * tau + 4):
                        for lh in range(2):
                            for hg in range(2):
                                chunks.append((mt, lh, hg))
                for lh in range(2):
                    for mt in range(12, 16):
                        for hg in range(2):
                            chunks.append((mt, lh, hg))

                LAG = 4
                pend = []

                def run_chunks(lo, hi, fills):
                    for i in range(lo, hi):
                        for th in fills.get(i, ()):
                            th()
                        pend.append((chunks[i], emit_S(chunks[i])))
                        if len(pend) > LAG:
                            c, ex = pend.pop(0)
                            emit_AV(c, ex)

                def flush_pend():
                    while pend:
                        c, ex = pend.pop(0)
                        emit_AV(c, ex)

                FILL_SLOTS = [0, 1, 2, 4, 5, 7, 8, 10, 11, 13, 14]

                with tc.tile_pool(name="scr", bufs=2, space="PSUM") as scr, \
                     tc.tile_pool(name="vtp", bufs=1, space="PSUM") as vtp:
                    # prologue: k-half of tile 0 + q heads 0-3; the v-half
                    # and remaining q heads ride as fills inside block 0
                    ths0 = kvconv_thunks(scr, vtp, 0)
                    for th in ths0[:6]:
                        th()
                    qproj_one(scr, 0)
                    qproj_one(scr, 1)
                    ths1 = kvconv_thunks(scr, vtp, 1)
                    fills0 = {
                        0: [ths0[6]], 1: [ths0[7]], 2: [ths0[8]],
                        3: [ths0[9]], 4: [ths0[10]],
                        5: [lambda: (qproj_one(scr, 2), qproj_one(scr, 3))],
                        6: [ths1[0]], 7: [ths1[1]], 8: [ths1[2]],
                        9: [ths1[3]], 10: [ths1[4]], 11: [ths1[5]],
                        12: [ths1[6]], 13: [ths1[7]], 14: [ths1[8]],
                        15: [ths1[9], ths1[10]],
                    }
                    run_chunks(0, 16, fills0)
                    # blocks 1-2 with kvconv fills for t+1
                    for tau in range(1, 3):
                        ths = kvconv_thunks(scr, vtp, tau + 1)
                        fills = {tau * 16 + s: [ths[j]]
                                 for j, s in enumerate(FILL_SLOTS)}
                        run_chunks(tau * 16, tau * 16 + 16, fills)

                with tc.tile_pool(name="trp", bufs=2, space="PSUM") as trp:
                    run_chunks(48, 64, {})
                    flush_pend()
                    nc.vector.reciprocal(zr[:, :], zp[:, :])
                    norm_lh(0)
                    norm_lh(1)
                    tr_lh(0, trp)
                    tr_lh(1, trp)

                with tc.tile_pool(name="ypp", bufs=2, space="PSUM") as ypp:
                    y_m(0, 0, ypp)
                    y_m(1, 0, ypp)
                    y_m(2, 0, ypp)
                    y_m(3, 0, ypp)
                    y_m(0, 1, ypp, nc.sync)
                    y_m(1, 1, ypp, nc.scalar)
                    y_m(2, 1, ypp, nc.sync)
                    y_m(3, 1, ypp, nc.scalar)

    nc.finalize()
    return nc


def _get_program():
    if "nc" not in _NC_CACHE:
        _NC_CACHE["nc"] = _build_program()
    return _NC_CACHE["nc"]


def _host_prep(x, wq, bq, wkv, bkv, dw_kernel, dw_bias, wo, bo):
    """Build the 8 per-core input maps (bigb bf16 + bigf f32)."""
    x = np.ascontiguousarray(np.asarray(x, np.float32))
    wq = np.asarray(wq, np.float32)
    wkv = np.asarray(wkv, np.float32)
    wo = np.asarray(wo, np.float32)
    bq = np.asarray(bq, np.float32)
    bkv = np.asarray(bkv, np.float32)
    dw_bias = np.asarray(dw_bias, np.float32)
    bo = np.asarray(bo, np.float32)
    dww = np.asarray(dw_kernel, np.float32).reshape(9, CH).T.copy()  # [1024, 9]

    # bias plane: dw_bias + bkv * sum(valid taps), SAME padding aware
    oy = np.arange(KH)
    valid_y = (2 * oy[:, None] + np.arange(3)[None, :]) < H      # [16, 3]
    valid_x = valid_y.copy()
    wsum = np.zeros((CH, KH, KW), np.float32)
    for tap in range(9):
        dy, dx = tap // 3, tap % 3
        m2 = np.outer(valid_y[:, dy], valid_x[:, dx]).astype(np.float32)
        wsum += dww[:, tap][:, None, None] * m2[None, :, :]
    bpl = (dw_bias[:, None] + bkv[:, None] * wsum.reshape(CH, NS)).astype(np.float32)

    # ---- shared f32r template (np.float32; PE rounds internally) ----
    tmpl = np.zeros((128, NR), np.float32)
    wkvb = wkv.reshape(4, 128, 8, 128)                         # [k, p, c, cc]
    for k in range(4):
        tmpl[:, k * 768:k * 768 + 128] = wkvb[k, :, 0, :]
        tmpl[:, k * 768 + 128:k * 768 + 256] = wkvb[k, :, 4, :]
    for c in (1, 5, 2, 6, 3, 7):
        tmpl[:, OFF_WKV[c]:OFF_WKV[c] + 512] = \
            wkvb[:, :, c, :].transpose(1, 0, 2).reshape(128, 512)

    bigb0 = np.zeros((128, NBF), BFNP)
    wqb = wq.reshape(4, 128, 4, 128)                           # [k, p, t, cc]
    for t in range(4):
        bigb0[:, OFF_WQ + t * 512:OFF_WQ + (t + 1) * 512] = \
            wqb[:, :, t, :].transpose(1, 0, 2).reshape(128, 512).astype(BFNP)
    wob = wo.reshape(4, 128, 4, 128)
    for m in range(4):
        bigb0[:, OFF_WO + m * 512:OFF_WO + (m + 1) * 512] = \
            wob[:, :, m, :].transpose(1, 0, 2).reshape(128, 512).astype(BFNP)

    bigf = np.zeros((128, NF32), np.float32)
    bigf[:, 0:4] = bq.reshape(4, 128).T
    bigf[:, 4:8] = bo.reshape(4, 128).T
    for c in range(8):
        bigf[:, OFF_DWW + c * 9:OFF_DWW + (c + 1) * 9] = \
            dww[c * 128:(c + 1) * 128, :]
        bigf[:, OFF_BPL[c]:OFF_BPL[c] + 256] = bpl[c * 128:(c + 1) * 128, :]

    in_maps = []
    for core in range(8):
        b, j = core // 4, core % 4
        xtb = x[b].reshape(L, DIM).T                            # [512, 1024]
        br = tmpl.copy()
        xa = xtb.reshape(4, 128, 2, 512)                        # [k, p, n, t']
        for k in range(4):
            br[:, k * 768 + 256:(k + 1) * 768] = xa[k, :, 0, :]
        br[:, OFF_XT1:OFF_XT1 + 2048] = \
            xa[:, :, 1, :].transpose(1, 0, 2).reshape(128, 2048)
        bigb = bigb0.copy()
        xc = xtb[:, j * 256:(j + 1) * 256].reshape(4, 128, 256)
        bigb[:, OFF_XTC:OFF_XTC + 1024] = \
            xc.transpose(1, 0, 2).reshape(128, 1024).astype(BFNP)
        in_maps.append({"bigr": br, "bigb": bigb, "bigf": bigf})
    return in_maps


def kernel(**inputs) -> np.ndarray:
    nc = _get_program()
    in_maps = _host_prep(**inputs)
    res = run_bass_kernel_spmd(nc, in_maps, core_ids=list(range(8)))
    out = np.zeros((B, H, W, DIM), np.float32)
    flat = out.reshape(B, L, DIM)
    for c in range(8):
        b, j = c // 4, c % 4
        flat[b, j * 256:(j + 1) * 256, :] = res.results[c]["y"].T
    return out


# revision 72
# speedup vs baseline: 1.1027x; 1.0128x over previous
"""MobileMQA Trainium2 kernel (8 NeuronCores, SPMD).

Reference computation (per batch b of 2):
  q  = x @ wq + bq                         [1024 tok, 512]
  kv = x @ wkv + bkv                       [1024 tok, 1024]
  kv = depthwise3x3_s2_same(kv) + dw_bias  [256 sp, 1024]
  k, v = split(kv)  -> reshape to shared-KV length M=2048 (channel fold)
  attn = softmax(q @ k^T * 0.125); out = attn @ v
  y = out @ wo + bo

Sharding: core c handles batch b=c//4, query chunk j=c%4 (256 tokens).
KV path (proj+conv) is replicated across the 4 cores of a batch (MQA).

Design notes (cost-model driven):
- All matmul moving operands are bf16 (1.0 cycles/row at any size; fp32r
  pays 4x below 256 rows). PSUM accumulation stays fp32.
- attn@V is computed with exp-scores as the STATIONARY operand and V as
  the moving one: out[l, d] per head accumulates over 16 m-tiles at 64
  moving rows each (16.4k PE cycles vs 32.8k the other way around).
  Softmax denominators come from an extra ones-column matmul per head.
- Depthwise conv runs on PE as 9 diagonal-weight matmuls per 128-channel
  tile, using per-tap valid-rectangle access patterns (no zero-padding
  pass). Diagonal weight matrices are prebuilt on the host.
- All inputs are packed into one bf16 + one f32 DRAM tensor, DMA'd in a
  handful of large chunks ordered by first use (HWDGE issue costs ~650ns
  per dma_start, so few large DMAs beat many small ones).
- Attention is software-pipelined into the kv/conv phase: score matmuls
  for m-tile group t interleave with projection/conv matmuls of group
  t+1, keeping ACT (exp) busy from ~8us onward.
"""
import os
import sys

for _p in ("/opt/trn_rl_repo", "/opt/trn_rl_repo/concourse"):
    if _p not in sys.path:
        sys.path.insert(0, _p)

_TRUNC = int(os.environ.get("KTRUNC", "9"))

import numpy as np
import ml_dtypes

import concourse.bass as bass
import concourse.mybir as mybir
import concourse.tile as tile
from concourse import bacc
from concourse.bass_utils import run_bass_kernel_spmd
from concourse.masks import make_identity

F32 = mybir.dt.float32
F32R = mybir.dt.float32r
BF16 = mybir.dt.bfloat16
AF = mybir.ActivationFunctionType
ALU = mybir.AluOpType
BFNP = ml_dtypes.bfloat16

DIM = 512
NH = 8
HD = 64
B, H, W = 2, 32, 32
L = H * W            # 1024 tokens per batch
KH = KW = 16
NS = KH * KW         # 256 conv-output spatial positions
M = NS * NH          # 2048 shared-KV positions
CH = 2 * DIM         # 1024 kv channels
SCALE = HD ** -0.5   # 0.125

# ---- f32r mega-tensor column layout (segments ordered by first use) ----
# ch-tile processing order: k0 v0 k1 v1 k2 v2 k3 v3 -> c = 0,4,1,5,2,6,3,7
# cols 0..3072 are k-interleaved [wkv-c0-k | wkv-c4-k | xT-n0-k] so the
# first kv matmul fires as soon as the k0 block lands
_CORD = [0, 4, 1, 5, 2, 6, 3, 7]
OFF_WKV = {1: 5120, 5: 5632, 2: 6144, 6: 6656, 3: 7168, 7: 7680}
OFF_XT1 = 3072                 # xT n1: [4k, 512]
NR = 8192

# conv taps: dy=0,1 run on PE (diag matmuls), dy=2 on DVE (stt chain)
_PE_TAPS = [(0, 0), (0, 1), (0, 2), (1, 0), (1, 1), (1, 2)]
_DVE_TAPS = [(2, 0), (2, 1), (2, 2)]

# f32 tensor: cst (bq 4, bo 4, 8 pad), dww [72], bias planes per ch-tile
OFF_DWW = 16
OFF_BPL = {c: 88 + i * 256 for i, c in enumerate(_CORD)}
NF32 = 88 + 8 * 256

# bf16 tensor: q path + wo
OFF_XTC = 0            # [4k, 256] = 1024
OFF_WQ = 1024          # [4t, 4k, 128] = 2048
OFF_WO = 3072          # [4m, 4k, 128] = 2048
NBF = 5120

PADW = 33
NPAD = PADW * PADW   # 1089

_NC_CACHE = {}


def _build_program():
    nc = bacc.Bacc(None)

    bigr_d = nc.dram_tensor("bigr", [128, NR], F32R, kind="ExternalInput")
    bigb_d = nc.dram_tensor("bigb", [128, NBF], BF16, kind="ExternalInput")
    bigf_d = nc.dram_tensor("bigf", [128, NF32], F32, kind="ExternalInput")
    y_d = nc.dram_tensor("y", [DIM, 256], F32, kind="ExternalOutput")

    with tile.TileContext(nc) as tc:
        with tc.tile_pool(name="wp", bufs=1) as wp, \
             tc.tile_pool(name="kvsbp", bufs=2) as kvsbp, \
             tc.tile_pool(name="vsbp", bufs=2) as vsbp, \
             tc.tile_pool(name="accp", bufs=2) as accp, \
             tc.tile_pool(name="expp", bufs=6) as expp:

            bigr = wp.tile([128, NR], F32R, tag="bigr")
            bigb = wp.tile([128, NBF], BF16, tag="bigb")
            bigf = wp.tile([128, NF32], F32, tag="bigf")

            def dma_r(lo, hi):
                nc.sync.dma_start(out=bigr[:, lo:hi], in_=bigr_d[:, lo:hi])

            def dma_b(lo, hi):
                nc.sync.dma_start(out=bigb[:, lo:hi], in_=bigb_d[:, lo:hi])

            def dma_f(lo, hi):
                nc.sync.dma_start(out=bigf[:, lo:hi], in_=bigf_d[:, lo:hi])

            for k in range(4):      # [wkv c0c4 | xT n0] per k-slice
                dma_r(k * 768, (k + 1) * 768)
            dma_f(0, 88)            # cst + dww (feeds Pool diag build)
            dma_r(3072, 5120)       # xT n1
            dma_b(0, 2048)          # xTc + wq t0,t1
            dma_f(88, 600)          # bpl c0, c4
            dma_b(2048, 3072)       # wq t2,t3
            dma_r(5120, 6144)       # wkv c1, c5
            dma_f(600, 1112)        # bpl c1, c5
            dma_r(6144, 7168)       # wkv c2, c6
            dma_f(1112, NF32)       # bpl rest
            dma_r(7168, NR)         # wkv c3, c7
            dma_b(3072, NBF)        # wo

            identf = wp.tile([128, 128], F32, tag="identf")
            make_identity(nc, identf)
            identb = wp.tile([128, 128], BF16, tag="identb")
            nc.vector.tensor_copy(identb[:, :], identf[:, :])
            onesf = wp.tile([128, 1], F32, tag="onesf")
            nc.vector.memset(onesf, 1.0)
            ones1 = wp.tile([128, 1], BF16, tag="ones1")
            nc.vector.tensor_copy(ones1[:, :], onesf[:, :])
            zpad = wp.tile([128, PADW], F32, tag="zpad")
            nc.vector.memset(zpad, 0.0)
            # diagonal conv-weight matrices, built on DVE with one
            # broadcast tensor_tensor per ch-tile (ident x tap-weight)
            diagr = wp.tile([128, 72 * 128], F32R, tag="diagr")

            def diag_build(c):
                # on GPSIMD: the Pool engine is otherwise idle
                for j in range(9):
                    o = (c * 9 + j) * 128
                    nc.gpsimd.tensor_scalar_mul(
                        diagr[:, o:o + 128], identf[:, :],
                        bigf[:, OFF_DWW + c * 9 + j:OFF_DWW + c * 9 + j + 1])
            # preload the exp ACT table during the DMA window
            warm = wp.tile([1, 1], F32, tag="warm")
            nc.vector.memset(warm, 0.0)
            nc.scalar.activation(warm[:, :], warm[:, :], AF.Exp)

            kT2 = wp.tile([64, M], F32R, tag="kT2")
            qT2 = wp.tile([64, M], F32R, tag="qT2")
            vaug = wp.tile([128, 16 * HD], BF16, tag="vaug")
            attn_sb = wp.tile([128, 2, 512], BF16, tag="attn_sb")
            attnT = wp.tile([128, 4, 256], BF16, tag="attnT")
            zr = wp.tile([128, 16], F32, tag="zr")
            ysb = wp.tile([128, 4, 256], F32, tag="ysb")

            def wkv_l(c, k):
                if c == 0:
                    o = k * 768
                elif c == 4:
                    o = k * 768 + 128
                else:
                    o = OFF_WKV[c] + k * 128
                return bigr[:, o:o + 128]

            def xt_r(n, k):
                o = (k * 768 + 256) if n == 0 else (OFF_XT1 + k * 512)
                return bigr[:, o:o + 512]

            def diag_l(c, j):
                o = (c * 9 + j) * 128
                return diagr[:, o:o + 128]

            def bpl_v(c):
                o = OFF_BPL[c]
                return bigf[:, o:o + 256]

            with tc.tile_pool(name="stp", bufs=2, space="PSUM") as stp, \
                 tc.tile_pool(name="avpp", bufs=1, space="PSUM") as avpp, \
                 tc.tile_pool(name="zpp", bufs=1, space="PSUM") as zpp:

                avp = [avpp.tile([128, 512], F32, tag=f"avp{l}",
                                 name=f"avp{l}") for l in range(2)]
                zp = zpp.tile([128, 16], F32, tag="zp")

                # ---------- kv proj + conv closures (PSUM pool passed in) ----------
                def new_kvsb(name):
                    """Zero-padded 33x33 conv-input layout; the SAME-pad
                    column (32) and bottom row (32) are zeroed on Pool."""
                    kvsb = kvsbp.tile([128, NPAD], F32R, tag="kvsb", name=name)
                    kb = kvsb[:, :]
                    pad_col = bass.AP(tensor=kb.tensor, offset=kb.offset + 32,
                                      ap=[kb.ap[0], [PADW, PADW]])
                    nc.vector.tensor_copy(pad_col, zpad[:, :])
                    nc.vector.tensor_copy(kvsb[:, PADW * 32:PADW * 32 + 32],
                                          zpad[:, 0:32])
                    return kvsb

                def kv_chunk(scr, c, n, kvsb):
                    kvp = scr.tile([128, 512], F32, tag="scr", name="kvp")
                    for k in range(4):
                        nc.tensor.matmul(kvp[:, :], wkv_l(c, k), xt_r(n, k),
                                         start=(k == 0), stop=(k == 3))
                    # 512 tokens = 16 padded rows of 32
                    kb = kvsb[:, :]
                    dst = bass.AP(tensor=kb.tensor,
                                  offset=kb.offset + n * 16 * PADW,
                                  ap=[kb.ap[0], [PADW, 16], [1, 32]])
                    nc.vector.tensor_copy(dst, kvp[:, :])

                def _win(kvsb, dy, dx):
                    kb = kvsb[:, :]
                    return bass.AP(tensor=kb.tensor,
                                   offset=kb.offset + PADW * dy + dx,
                                   ap=[kb.ap[0], [2 * PADW, KH], [2, KW]])

                def conv_dve(c, kvsb):
                    # dy=2 tap row on DVE: acc = bias-plane + 3 taps
                    acc = accp.tile([128, 256], F32, tag="acc")
                    av = acc[:, :].rearrange("p (a b) -> p a b", b=KW)
                    for i, (dy, dx) in enumerate(_DVE_TAPS):
                        prev = bpl_v(c)[:, :].rearrange(
                            "p (a b) -> p a b", b=KW) if i == 0 else av
                        nc.vector.scalar_tensor_tensor(
                            av, _win(kvsb, dy, dx),
                            bigf[:, OFF_DWW + c * 9 + 3 * dy + dx:
                                 OFF_DWW + c * 9 + 3 * dy + dx + 1],
                            prev, op0=ALU.mult, op1=ALU.add)
                    return acc

                def conv_taps(scr, c, kvsb, taps, cvp=None):
                    if cvp is None:
                        cvp = scr.tile([128, 512], F32, tag="scr", name="cvp")
                    for dy, dx in taps:
                        nc.tensor.matmul(cvp[:, 0:256], diag_l(c, 3 * dy + dx),
                                         _win(kvsb, dy, dx),
                                         start=((dy, dx) == _PE_TAPS[0]),
                                         stop=((dy, dx) == _PE_TAPS[-1]))
                    return cvp

                def k_finish(t, cvp, acc):
                    # PE conv partial + DVE partial (incl bias) -> kT2
                    for gi in range(2):
                        g = 2 * t + gi
                        nc.vector.tensor_tensor(
                            kT2[:, g * 256:(g + 1) * 256],
                            cvp[gi * 64:(gi + 1) * 64, 0:256],
                            acc[gi * 64:(gi + 1) * 64, :], op=ALU.add)

                def v_finish(t, cvp, acc, vtp):
                    # transpose [128 ch, 128 s] -> [128 s, 128 ch]; the two
                    # 64-wide ch-groups then scatter to their m-tiles
                    vsb = vsbp.tile([128, 256], BF16, tag="vsb")
                    nc.vector.tensor_tensor(
                        vsb[:, :], cvp[:, 0:256], acc[:, :], op=ALU.add)
                    for sh in range(2):
                        vt = vtp.tile([128, 128], BF16, tag="vt")
                        nc.tensor.matmul(vt[:, :],
                                         vsb[:, sh * 128:(sh + 1) * 128],
                                         identb[:, :], is_transpose=True)
                        av = vaug[:, :]
                        dst = bass.AP(tensor=av.tensor,
                                      offset=av.offset + t * 256 + sh * 64,
                                      ap=[av.ap[0], [128, 2], [1, 64]])
                        nc.vector.tensor_copy(dst, vt[:, :])

                def qproj_one(scr, t):
                    qp = scr.tile([128, 512], F32, tag="scr", name="qp")
                    for k in range(4):
                        nc.tensor.matmul(
                            qp[:, 0:256],
                            bigb[:, OFF_WQ + t * 512 + k * 128:
                                 OFF_WQ + t * 512 + (k + 1) * 128],
                            bigb[:, OFF_XTC + k * 256:OFF_XTC + (k + 1) * 256],
                            start=(k == 0), stop=(k == 3))
                    for gi in range(2):
                        h = 2 * t + gi
                        nc.vector.tensor_scalar_add(
                            qT2[:, h * 256:(h + 1) * 256],
                            qp[gi * 64:(gi + 1) * 64, 0:256],
                            bigf[gi * 64:(gi + 1) * 64, t:t + 1])

                def kvconv_thunks(scr, vtp, t):
                    """10 thunks: k-tile t then v-tile t."""
                    st8 = {}

                    def kt_chunk(n):
                        if "k" not in st8:
                            st8["k"] = new_kvsb("kvsbk")
                        kv_chunk(scr, t, n, st8["k"])

                    def kt_conv(lo):
                        if "kc" not in st8:
                            st8["ka"] = conv_dve(t, st8["k"])
                            st8["kc"] = conv_taps(scr, t, st8["k"],
                                                  _PE_TAPS[:3])
                        else:
                            conv_taps(scr, t, st8["k"], _PE_TAPS[3:],
                                      st8["kc"])

                    def vt_chunk(n):
                        if "v" not in st8:
                            st8["v"] = new_kvsb("kvsbv")
                        kv_chunk(scr, 4 + t, n, st8["v"])

                    def vt_conv(lo):
                        if "vc" not in st8:
                            st8["va"] = conv_dve(4 + t, st8["v"])
                            st8["vc"] = conv_taps(scr, 4 + t, st8["v"],
                                                  _PE_TAPS[:3])
                        else:
                            conv_taps(scr, 4 + t, st8["v"], _PE_TAPS[3:],
                                      st8["vc"])

                    return [
                        lambda: (diag_build(t), diag_build(4 + t)),
                        lambda: kt_chunk(0),
                        lambda: kt_chunk(1),
                        lambda: kt_conv(0),
                        lambda: kt_conv(4),
                        lambda: k_finish(t, st8["kc"], st8["ka"]),
                        lambda: vt_chunk(0),
                        lambda: vt_chunk(1),
                        lambda: vt_conv(0),
                        lambda: vt_conv(4),
                        lambda: (v_finish(t, st8["vc"], st8["va"], vtp)),
                    ]

                # ---------- attention chunk emission ----------
                qv = qT2[:, :].rearrange("p (h l) -> p h l", l=256)

                def emit_S(c):
                    mt, lh, hg = c
                    st = stp.tile([128, 512], F32, tag="st")
                    rhs = qv[:, hg * 4:(hg + 1) * 4, lh * 128:(lh + 1) * 128]
                    nc.tensor.matmul(st[:, :], kT2[:, mt * 128:(mt + 1) * 128],
                                     rhs, start=True, stop=True)
                    ex = expp.tile([128, 512], BF16, tag="ex")
                    nc.scalar.activation(ex[:, :], st[:, :], AF.Exp,
                                         scale=float(SCALE))
                    return ex

                def emit_AV(c, ex):
                    # One PSUM accumulation group per bank: the first matmul
                    # start=True lazily zeroes the whole 2KB region; each
                    # head's first write consumes its share of the zeroing.
                    mt, lh, hg = c
                    for hi in range(4):
                        h = hg * 4 + hi
                        exh = ex[:, hi * 128:(hi + 1) * 128]
                        nc.tensor.matmul(avp[lh][:, h * 64:(h + 1) * 64],
                                         exh, vaug[:, mt * 64:(mt + 1) * 64],
                                         start=(mt == 0 and h == 0),
                                         stop=(mt == 15 and h == 7))
                        nc.tensor.matmul(zp[:, lh * 8 + h:lh * 8 + h + 1],
                                         exh, ones1[:, :],
                                         start=(mt == 0 and lh == 0 and h == 0),
                                         stop=(mt == 15 and lh == 1 and h == 7))

                # ---------- tail closures ----------
                def norm_lh(lh):
                    for h in range(NH):
                        nc.vector.tensor_scalar_mul(
                            attn_sb[:, lh, h * 64:(h + 1) * 64],
                            avp[lh][:, h * 64:(h + 1) * 64],
                            zr[:, lh * 8 + h:lh * 8 + h + 1])

                def tr_lh(lh, trp):
                    for kk in range(4):
                        trt = trp.tile([128, 128], BF16, tag="trp")
                        nc.tensor.matmul(
                            trt[:, :],
                            attn_sb[:, lh, kk * 128:(kk + 1) * 128],
                            identb[:, :], is_transpose=True)
                        nc.vector.tensor_copy(
                            attnT[:, kk, lh * 128:(lh + 1) * 128], trt[:, :])

                def y_m(m, lh, ypp, dma_engine):
                    yp = ypp.tile([128, 256], F32, tag="yp")
                    for k in range(4):
                        nc.tensor.matmul(
                            yp[:, 0:128],
                            bigb[:, OFF_WO + m * 512 + k * 128:
                                 OFF_WO + m * 512 + (k + 1) * 128],
                            attnT[:, k, lh * 128:(lh + 1) * 128],
                            start=(k == 0), stop=(k == 3))
                    nc.vector.tensor_scalar_add(
                        ysb[:, m, lh * 128:(lh + 1) * 128], yp[:, 0:128],
                        bigf[:, 4 + m:5 + m])
                    dma_engine.dma_start(
                        out=y_d[m * 128:(m + 1) * 128,
                                lh * 128:(lh + 1) * 128],
                        in_=ysb[:, m, lh * 128:(lh + 1) * 128])

                # ---------- chunk schedule ----------
                # block 0 is hg-major (so only q heads 0-3 gate the start),
                # blocks 1-2 mt-major, block 3 lh-major (frees lh0 early)
                chunks = []
                for hg in range(2):
                    for mt in range(4):
                        for lh in range(2):
                            chunks.append((mt, lh, hg))
                for tau in range(1, 3):
                    for mt in range(4 * tau, 4 * tau + 4):
                        for lh in range(2):
                            for hg in range(2):
                                chunks.append((mt, lh, hg))
                for lh in range(2):
                    for mt in range(12, 16):
                        for hg in range(2):
                            chunks.append((mt, lh, hg))

                LAG = 4
                pend = []

                def run_chunks(lo, hi, fills):
                    for i in range(lo, hi):
                        for th in fills.get(i, ()):
                            th()
                        pend.append((chunks[i], emit_S(chunks[i])))
                        if len(pend) > LAG:
                            c, ex = pend.pop(0)
                            emit_AV(c, ex)

                def flush_pend():
                    while pend:
                        c, ex = pend.pop(0)
                        emit_AV(c, ex)

                FILL_SLOTS = [0, 1, 2, 4, 5, 7, 8, 10, 11, 13, 14]

                with tc.tile_pool(name="scr", bufs=2, space="PSUM") as scr, \
                     tc.tile_pool(name="vtp", bufs=1, space="PSUM") as vtp:
                    # prologue: k-half of tile 0 + q heads 0-3; the v-half
                    # and remaining q heads ride as fills inside block 0
                    ths0 = kvconv_thunks(scr, vtp, 0)
                    for th in ths0[:6]:
                        th()
                    qproj_one(scr, 0)
                    qproj_one(scr, 1)
                    ths1 = kvconv_thunks(scr, vtp, 1)
                    fills0 = {
                        0: [ths0[6]], 1: [ths0[7]], 2: [ths0[8]],
                        3: [ths0[9]], 4: [ths0[10]],
                        5: [lambda: (qproj_one(scr, 2), qproj_one(scr, 3))],
                        6: [ths1[0]], 7: [ths1[1]], 8: [ths1[2]],
                        9: [ths1[3]], 10: [ths1[4]], 11: [ths1[5]],
                        12: [ths1[6]], 13: [ths1[7]], 14: [ths1[8]],
                        15: [ths1[9], ths1[10]],
                    }
                    run_chunks(0, 16, fills0)
                    # blocks 1-2 with kvconv fills for t+1
                    for tau in range(1, 3):
                        ths = kvconv_thunks(scr, vtp, tau + 1)
                        fills = {tau * 16 + s: [ths[j]]
                                 for j, s in enumerate(FILL_SLOTS)}
                        run_chunks(tau * 16, tau * 16 + 16, fills)

                with tc.tile_pool(name="trp", bufs=2, space="PSUM") as trp:
                    run_chunks(48, 64, {})
                    flush_pend()
                    nc.vector.reciprocal(zr[:, :], zp[:, :])
                    norm_lh(0)
                    tr_lh(0, trp)
                    norm_lh(1)
                    tr_lh(1, trp)

                with tc.tile_pool(name="ypp", bufs=2, space="PSUM") as ypp:
                    y_m(0, 0, ypp, nc.sync)
                    y_m(1, 0, ypp, nc.scalar)
                    y_m(2, 0, ypp, nc.sync)
                    y_m(3, 0, ypp, nc.scalar)
                    y_m(0, 1, ypp, nc.sync)
                    y_m(1, 1, ypp, nc.scalar)
                    y_m(2, 1, ypp, nc.sync)
                    y_m(3, 1, ypp, nc.scalar)

    nc.finalize()
    return nc


def _get_program():
    if "nc" not in _NC_CACHE:
        _NC_CACHE["nc"] = _build_program()
    return _NC_CACHE["nc"]


def _host_prep(x, wq, bq, wkv, bkv, dw_kernel, dw_bias, wo, bo):
    """Build the 8 per-core input maps (bigb bf16 + bigf f32)."""
    x = np.ascontiguousarray(np.asarray(x, np.float32))
    wq = np.asarray(wq, np.float32)
    wkv = np.asarray(wkv, np.float32)
    wo = np.asarray(wo, np.float32)
    bq = np.asarray(bq, np.float32)
    bkv = np.asarray(bkv, np.float32)
    dw_bias = np.asarray(dw_bias, np.float32)
    bo = np.asarray(bo, np.float32)
    dww = np.asarray(dw_kernel, np.float32).reshape(9, CH).T.copy()  # [1024, 9]

    # bias plane: dw_bias + bkv * sum(valid taps), SAME padding aware
    oy = np.arange(KH)
    valid_y = (2 * oy[:, None] + np.arange(3)[None, :]) < H      # [16, 3]
    valid_x = valid_y.copy()
    wsum = np.zeros((CH, KH, KW), np.float32)
    for tap in range(9):
        dy, dx = tap // 3, tap % 3
        m2 = np.outer(valid_y[:, dy], valid_x[:, dx]).astype(np.float32)
        wsum += dww[:, tap][:, None, None] * m2[None, :, :]
    bpl = (dw_bias[:, None] + bkv[:, None] * wsum.reshape(CH, NS)).astype(np.float32)

    # ---- shared f32r template (np.float32; PE rounds internally) ----
    tmpl = np.zeros((128, NR), np.float32)
    wkvb = wkv.reshape(4, 128, 8, 128)                         # [k, p, c, cc]
    for k in range(4):
        tmpl[:, k * 768:k * 768 + 128] = wkvb[k, :, 0, :]
        tmpl[:, k * 768 + 128:k * 768 + 256] = wkvb[k, :, 4, :]
    for c in (1, 5, 2, 6, 3, 7):
        tmpl[:, OFF_WKV[c]:OFF_WKV[c] + 512] = \
            wkvb[:, :, c, :].transpose(1, 0, 2).reshape(128, 512)

    bigb0 = np.zeros((128, NBF), BFNP)
    wqb = wq.reshape(4, 128, 4, 128)                           # [k, p, t, cc]
    for t in range(4):
        bigb0[:, OFF_WQ + t * 512:OFF_WQ + (t + 1) * 512] = \
            wqb[:, :, t, :].transpose(1, 0, 2).reshape(128, 512).astype(BFNP)
    wob = wo.reshape(4, 128, 4, 128)
    for m in range(4):
        bigb0[:, OFF_WO + m * 512:OFF_WO + (m + 1) * 512] = \
            wob[:, :, m, :].transpose(1, 0, 2).reshape(128, 512).astype(BFNP)

    bigf = np.zeros((128, NF32), np.float32)
    bigf[:, 0:4] = bq.reshape(4, 128).T
    bigf[:, 4:8] = bo.reshape(4, 128).T
    for c in range(8):
        bigf[:, OFF_DWW + c * 9:OFF_DWW + (c + 1) * 9] = \
            dww[c * 128:(c + 1) * 128, :]
        bigf[:, OFF_BPL[c]:OFF_BPL[c] + 256] = bpl[c * 128:(c + 1) * 128, :]

    in_maps = []
    for core in range(8):
        b, j = core // 4, core % 4
        xtb = x[b].reshape(L, DIM).T                            # [512, 1024]
        br = tmpl.copy()
        xa = xtb.reshape(4, 128, 2, 512)                        # [k, p, n, t']
        for k in range(4):
            br[:, k * 768 + 256:(k + 1) * 768] = xa[k, :, 0, :]
        br[:, OFF_XT1:OFF_XT1 + 2048] = \
            xa[:, :, 1, :].transpose(1, 0, 2).reshape(128, 2048)
        bigb = bigb0.copy()
        xc = xtb[:, j * 256:(j + 1) * 256].reshape(4, 128, 256)
        bigb[:, OFF_XTC:OFF_XTC + 1024] = \
            xc.transpose(1, 0, 2).reshape(128, 1024).astype(BFNP)
        in_maps.append({"bigr": br, "bigb": bigb, "bigf": bigf})
    return in_maps


def kernel(**inputs) -> np.ndarray:
    nc = _get_program()
    in_maps = _host_prep(**inputs)
    res = run_bass_kernel_spmd(nc, in_maps, core_ids=list(range(8)))
    out = np.zeros((B, H, W, DIM), np.float32)
    flat = out.reshape(B, L, DIM)
    for c in range(8):
        b, j = c // 4, c % 4
        flat[b, j * 256:(j + 1) * 256, :] = res.results[c]["y"].T
    return out
